# revision 1
# baseline (speedup 1.0000x reference)
"""CISS-VAE (per-cluster MoE-routed MLP chain) Trainium2 kernel.

Strategy (routing done on host, compute on device):
  - Rows are grouped by cluster label on the host. Core c processes all rows
    of cluster c (C == n_cores == 8), so every GEMM on the device is a dense
    per-cluster GEMM — this removes the 8x redundant compute the reference
    does (einsum over all clusters then select).
  - All tensors on device are feature-major ([features, rows]): weights W are
    used directly as matmul lhsT ([f_in(K), f_out(M)]), activations are the
    moving operand ([K, rows_block]). x/eps are transposed on the host.
  - Matmul operands are bf16 (full PE rate + fast weight load), accumulation
    is fp32 in PSUM; per-feature biases live on partitions and are fused into
    the PSUM->SBUF eviction (Relu/Identity/Exp), split between the Scalar and
    Vector engines to balance load.
  - Row blocks are software-pipelined: the encoder of block b+1 is emitted
    before the decoder of block b so the PE never idles during the latent
    reparameterization (ACT/DVE) chain.
  - Weight DMAs are emitted just-in-time before their first use; x/weights go
    on the sync HWDGE queue, eps/bias on the scalar HWDGE queue, output
    stores on the gpsimd SWDGE queue.
"""

import ml_dtypes
import numpy as np

import concourse.bacc as bacc
import concourse.mybir as mybir
import concourse.tile as tile
from concourse import bass_utils

P = 128
D_IN, LAT, C = 512, 64, 8
H0, H1, H2 = 1024, 512, 256
N_CORES = 8
F32 = mybir.dt.float32
BF16 = mybir.dt.bfloat16
AF = mybir.ActivationFunctionType
ALU = mybir.AluOpType
BF16_NP = ml_dtypes.bfloat16


def _ceil_to(x, m):
    return ((x + m - 1) // m) * m


def _b2d(b):
    """[f] bias -> [128, n_mtiles] (partition-major per m-tile)."""
    f = b.shape[0]
    if f >= P:
        return np.ascontiguousarray(b.reshape(f // P, P).T.astype(np.float32))
    return np.ascontiguousarray(b.reshape(1, f).T.astype(np.float32))


# layer table: name -> (f_in, f_out)
LAYERS = dict(
    enc0=(D_IN, H0),
    encu=(H0, H1),
    enc2=(H1, H2),
    mu=(H2, LAT),
    lv=(H2, LAT),
    dec0=(LAT, H2),
    dec1=(H2, H1),
    dec2=(H1, H0),
    fin=(H0, D_IN),
)


def _build_module(npad, blocks):
    nc = bacc.Bacc("TRN2", target_bir_lowering=False, debug=False)

    dram = {}

    def din(name, shape, dt):
        dram[name] = nc.dram_tensor(name, list(shape), dt, kind="ExternalInput").ap()
        return dram[name]

    xT = din("xT", (D_IN, npad), BF16)
    epsT = din("epsT", (LAT, npad), F32)

    for name, (fi, fo) in LAYERS.items():
        din("w_" + name, (fi, fo), BF16)
        din("b_" + name, (P if fo >= P else fo, max(1, fo // P)), F32)

    outT = nc.dram_tensor("outT", [D_IN, npad], F32, kind="ExternalOutput").ap()

    with tile.TileContext(nc) as tc:
        with (
            tc.tile_pool(name="wpool", bufs=1) as wpool,
            tc.tile_pool(name="acts", bufs=2) as acts,
            tc.tile_pool(name="psum", bufs=7, space="PSUM") as psum,
        ):
            wsb = {}  # name -> list of [kp, f_out] tiles (loaded lazily)
            bsb = {}  # name -> [P or fo, n_m] tile
            dma_rr = [0]

            # The Scalar engine runs the critical-path PSUM evictions, and its
            # instruction queue is FIFO: a DMA issue that waits on a tile slot
            # would head-of-line-block evictions and stall the PE. So Scalar
            # only gets prologue DMAs (block-0 x / enc0 weights, which can
            # never wait), sync carries the steady-state loads, and gpsimd
            # (SWDGE) carries decoder weights and output stores.
            def prologue_dma(out, in_):
                eng = nc.sync if dma_rr[0] % 2 == 0 else nc.scalar
                dma_rr[0] += 1
                eng.dma_start(out, in_)

            DEC_W = ("dec0", "dec1", "dec2", "fin")

            def load_weights(name):
                if name in wsb:
                    return
                fi, fo = LAYERS[name]
                ktiles = []
                for k in range(max(1, fi // P)):
                    kp = min(P, fi)
                    w_t = wpool.tile([kp, fo], BF16, tag=f"w_{name}_{k}", name=f"w_{name}_{k}")
                    src_ap = dram["w_" + name][k * P : k * P + kp, :]
                    if name == "enc0":
                        prologue_dma(w_t[:], src_ap)
                    elif name in DEC_W:
                        nc.gpsimd.dma_start(w_t[:], src_ap)
                    else:
                        nc.sync.dma_start(w_t[:], src_ap)
                    ktiles.append(w_t)
                wsb[name] = ktiles
                bp = P if fo >= P else fo
                b_t = wpool.tile([bp, max(1, fo // P)], F32, tag=f"b_{name}", name=f"b_{name}")
                nc.gpsimd.dma_start(b_t[:], dram["b_" + name][:])
                bsb[name] = b_t

            def dense(lname, in_tiles, nb, func, scale=1.0, bufs=2, out_dt=BF16, evict="act", nsplit=1, ms=None):
                """out[f_out, nb] = func(scale*(W.T @ in) + b); returns m-tile list.

                evict="act": scalar-engine activation (any func).
                evict="dve": vector-engine tensor_scalar (Relu or Identity only).
                nsplit: split the rows dim into halves so thin layers pipeline
                their PSUM evictions with the following layer's matmuls.
                """
                load_weights(lname)
                fi, fo = LAYERS[lname]
                wk, bt = wsb[lname], bsb[lname]
                n_m = max(1, fo // P)
                n_k = len(wk)
                if nb % 512 or nsplit * 256 > nb:
                    nsplit = 1
                nh = nb // nsplit
                if ms is None:
                    ms = range(n_m)
                outs = []
                for m in ms:
                    mp = min(P, fo)
                    o_t = acts.tile(
                        [mp, nb], out_dt, tag=f"{lname}_{m}", bufs=bufs,
                        name=f"h_{lname}_{m}",
                    )
                    bias = bt[:mp, m : m + 1]
                    for h in range(nsplit):
                        sl = slice(h * nh, (h + 1) * nh)
                        ps = psum.tile([mp, nh], F32, tag="ps", name=f"ps_{lname}_{m}_{h}")
                        for k in range(n_k):
                            nc.tensor.matmul(
                                ps[:],
                                wk[k][:, m * mp : (m + 1) * mp],
                                in_tiles[k][:, sl],
                                start=(k == 0),
                                stop=(k == n_k - 1),
                            )
                        use_dve = evict == "dve" or (evict == "alt" and m % 2 == 1)
                        if use_dve:
                            if func is AF.Relu:
                                nc.vector.tensor_scalar(o_t[:, sl], ps[:], bias, 0.0, ALU.add, ALU.max)
                            else:  # Identity
                                nc.vector.tensor_scalar(o_t[:, sl], ps[:], bias, None, ALU.add)
                        else:
                            nc.scalar.activation(o_t[:, sl], ps[:], func, bias=bias, scale=scale)
                    outs.append(o_t)
                return outs

            n_blk = len(blocks)
            offs = [sum(blocks[:i]) for i in range(n_blk)]
            lat_out = [None] * n_blk  # z tiles per block
            x_in = [None] * n_blk
            eps_in = [None] * n_blk
            enc_out = [None] * n_blk

            def stage_load(b):
                nb, off = blocks[b], offs[b]
                x_tiles = []
                for k in range(D_IN // P):
                    x_t = acts.tile([P, nb], BF16, tag=f"x_{k}", bufs=3, name=f"x_{k}")
                    src_ap = xT[k * P : (k + 1) * P, off : off + nb]
                    if b == 0:
                        prologue_dma(x_t[:], src_ap)
                    else:
                        nc.sync.dma_start(x_t[:], src_ap)
                    x_tiles.append(x_t)
                e_t = acts.tile([LAT, nb], F32, tag="eps", bufs=3, name="e_t")
                (prologue_dma if b == 0 else nc.sync.dma_start)(e_t[:], epsT[:, off : off + nb])
                x_in[b], eps_in[b] = x_tiles, e_t

            h3_of = [None] * n_blk
            h4_of = [None] * n_blk

            def stage_encA(b):
                nb = blocks[b]
                h0 = dense("enc0", x_in[b], nb, AF.Relu, evict="alt")
                h1 = dense("encu", h0, nb, AF.Relu, evict="alt")
                enc_out[b] = dense("enc2", h1, nb, AF.Relu, evict="alt")

            def stage_mu(b):
                nb = blocks[b]
                h2 = enc_out[b]
                mu = dense("mu", h2, nb, AF.Identity, out_dt=F32, evict="dve")[0]
                sg = dense("lv", h2, nb, AF.Exp, scale=0.5, out_dt=F32)[0]
                enc_out[b] = (mu, sg)

            def stage_lat(b):
                nb = blocks[b]
                mu, sg = enc_out[b]
                tmp = acts.tile([LAT, nb], F32, tag="tmp", bufs=2, name="tmp")
                nc.vector.tensor_mul(tmp[:], sg[:], eps_in[b][:])
                z = acts.tile([LAT, nb], BF16, tag="z", bufs=2, name="z")
                nc.vector.tensor_add(z[:], tmp[:], mu[:])
                lat_out[b] = z

            def stage_dec0(b):
                h3_of[b] = dense("dec0", [lat_out[b]], blocks[b], AF.Relu, evict="alt")

            def stage_dec1(b):
                h4_of[b] = dense("dec1", h3_of[b], blocks[b], AF.Relu, evict="alt")

            def stage_dec2(b):
                nb, off = blocks[b], offs[b]
                h5 = dense("dec2", h4_of[b], nb, AF.Relu, evict="alt")
                ot = dense("fin", h5, nb, AF.Identity, out_dt=F32, evict="alt")
                for m in range(D_IN // P):
                    nc.sync.dma_start(outT[m * P : (m + 1) * P, off : off + nb], ot[m][:])

            # Warm up the PE (HAM clock gate) with dummy matmuls while the
            # prologue DMAs stream in: real matmuls then start at 2.4 GHz.
            wu_w = wpool.tile([P, P], BF16, tag="wu_w", name="wu_w")
            wu_x = wpool.tile([P, 512], BF16, tag="wu_x", name="wu_x")
            nc.vector.memset(wu_w[:], 0.0)
            nc.vector.memset(wu_x[:], 0.0)
            wu_ps = psum.tile([P, 512], F32, tag="wu_ps", bufs=1, name="wu_ps")
            for _ in range(20):
                nc.tensor.matmul(wu_ps[:], wu_w[:], wu_x[:], start=True, stop=True)

            # software pipeline: decoder stages of block b-1 are interleaved
            # between the thin encoder stages of block b so the PE always has
            # matmul work while PSUM evictions / the latent chain complete.
            stage_load(0)
            stage_encA(0)
            stage_mu(0)
            stage_lat(0)
            for b in range(1, n_blk):
                stage_load(b)
                stage_encA(b)
                stage_dec0(b - 1)
                stage_mu(b)
                stage_lat(b)
                stage_dec1(b - 1)
                stage_dec2(b - 1)
            stage_dec0(n_blk - 1)
            stage_dec1(n_blk - 1)
            stage_dec2(n_blk - 1)

    nc.compile()
    return nc


def kernel(**inputs):
    x = np.asarray(inputs["x"], dtype=np.float32)
    lbl = np.asarray(inputs["cluster_labels"]).astype(np.int64)
    eps = np.asarray(inputs["eps"], dtype=np.float32)
    B = x.shape[0]

    counts = np.bincount(lbl, minlength=C)
    npad = max(512, _ceil_to(int(counts.max()), 64))
    n_full, rem = divmod(npad, 512)
    blocks = [512] * n_full + ([rem] if rem else [])

    rows = [np.nonzero(lbl == c)[0] for c in range(C)]

    def w16(a):
        return np.ascontiguousarray(np.asarray(a, dtype=np.float32).astype(BF16_NP))

    shared = {
        "w_enc0": w16(inputs["enc_W0"]),
        "b_enc0": _b2d(np.asarray(inputs["enc_b0"])),
        "w_enc2": w16(inputs["enc_W2"]),
        "b_enc2": _b2d(np.asarray(inputs["enc_b2"])),
        "w_mu": w16(inputs["mu_W"]),
        "b_mu": _b2d(np.asarray(inputs["mu_b"])),
        "w_lv": w16(inputs["lv_W"]),
        "b_lv": _b2d(0.5 * np.asarray(inputs["lv_b"])),
        "w_dec1": w16(inputs["dec_W1"]),
        "b_dec1": _b2d(np.asarray(inputs["dec_b1"])),
    }

    in_maps = []
    for c in range(C):
        r = rows[c]
        xT = np.zeros((D_IN, npad), BF16_NP)
        xT[:, : len(r)] = x[r].T.astype(BF16_NP)
        epsT = np.zeros((LAT, npad), np.float32)
        epsT[:, : len(r)] = eps[r].T
        m = dict(shared)
        m["xT"] = xT
        m["epsT"] = epsT
        m["w_encu"] = w16(inputs["enc_Wu"][c])
        m["b_encu"] = _b2d(np.asarray(inputs["enc_bu"][c]))
        m["w_dec0"] = w16(inputs["dec_Wu0"][c])
        m["b_dec0"] = _b2d(np.asarray(inputs["dec_bu0"][c]))
        m["w_dec2"] = w16(inputs["dec_Wu2"][c])
        m["b_dec2"] = _b2d(np.asarray(inputs["dec_bu2"][c]))
        m["w_fin"] = w16(inputs["fin_W"][c])
        m["b_fin"] = _b2d(np.asarray(inputs["fin_b"][c]))
        in_maps.append(m)

    nc = _build_module(npad, blocks)
    res = bass_utils.run_bass_kernel_spmd(nc, in_maps, core_ids=list(range(N_CORES)))
    global LAST_RESULTS
    LAST_RESULTS = res

    out = np.empty((B, D_IN), np.float32)
    for c in range(C):
        r = rows[c]
        out[r] = res.results[c]["outT"][:, : len(r)].T
    return out



# revision 2
# speedup vs baseline: 1.1769x; 1.1769x over previous
"""CISS-VAE (per-cluster MoE-routed MLP chain) Trainium2 kernel.

Strategy (routing on host, compute on device):
  - Rows are grouped by cluster label on the host; core c processes all rows
    of cluster c (C == n_cores == 8), so every GEMM is a dense per-cluster
    GEMM (no 8x redundant einsum like the reference).
  - The encoder (enc0, encu, enc2, fused mu|lv head) runs in fp8-e4m3 with
    DoubleRow matmuls (2 fp8 MACs/cell/cycle): the VAE latent z is dominated
    by the eps noise term, so encoder-side quantization error is attenuated
    ~10x below the decoder's sensitivity (measured end-to-end rel err 1.9e-3
    vs 1.8e-3 all-bf16). The decoder stays bf16: decoder-side fp8 measured
    1.2e-2..3.3e-2, too close to the 2e-2 gate.
  - fp8 scaling: weights x32, x input x16, hidden activations x8 (power-of-2
    scales folded exactly into the PSUM-eviction scale and host-scaled
    biases); keeps fp8 operands out of the subnormal range.
  - Schedule is a layer-wavefront: each layer runs over all row blocks
    before the next layer starts, so a block's PSUM evictions always overlap
    the other blocks' matmuls and the PE never drains between layers. The
    thin enc2/mulv layers are staggered so the latent (mu/sigma/z) chain of
    block b hides under enc2 matmuls of blocks b+2..; npad is split into
    near-equal blocks (<=512 for one PSUM bank), largest first so the tail
    block is smallest.
  - Per-feature biases live on partitions and are fused into the PSUM->SBUF
    eviction: encoder evictions on the Scalar engine (needs scale+bias+func
    in one op), decoder evictions and the latent chain on the Vector engine.
  - Weight/x DMAs are emitted just-in-time (sync HWDGE queue, prologue split
    with scalar); eps, biases and decoder weights go on the gpsimd SWDGE
    queue; output stores on sync.
"""

import ml_dtypes
import numpy as np

import concourse.bacc as bacc
import concourse.mybir as mybir
import concourse.tile as tile
from concourse import bass_utils

P = 128
D_IN, LAT, C = 512, 64, 8
H0, H1, H2 = 1024, 512, 256
N_CORES = 8
F32 = mybir.dt.float32
BF16 = mybir.dt.bfloat16
F8 = mybir.dt.float8e4
AF = mybir.ActivationFunctionType
ALU = mybir.AluOpType
DR = mybir.MatmulPerfMode.DoubleRow
BF16_NP = ml_dtypes.bfloat16
F8_NP = ml_dtypes.float8_e4m3

S_X = 16.0  # fp8 scale on the x input
S_W = 32.0  # fp8 scale on encoder weights
S_A = 8.0   # fp8 scale on encoder hidden activations

# encoder layers (fp8 DoubleRow): name -> (f_in, f_out)
ENC_LAYERS = dict(enc0=(D_IN, H0), encu=(H0, H1), enc2=(H1, H2), mulv=(H2, 2 * LAT))
# decoder layers (bf16): name -> (f_in, f_out)
DEC_LAYERS = dict(dec0=(LAT, H2), dec1=(H2, H1), dec2=(H1, H0), fin=(H0, D_IN))


def _ceil_to(x, m):
    return ((x + m - 1) // m) * m


def _b2d(b):
    """[f] bias -> [min(f,128), n_mtiles] (partition-major per m-tile)."""
    b = np.asarray(b, np.float32)
    f = b.shape[0]
    if f >= P:
        return np.ascontiguousarray(b.reshape(f // P, P).T)
    return np.ascontiguousarray(b.reshape(1, f).T)


def _w8(w):
    """[fi, fo] weight -> fp8 DR layout [128, n_k2, 2, fo], scaled by S_W."""
    w = np.asarray(w, np.float32)
    fi, fo = w.shape
    n_k2 = max(1, fi // 256)
    a = (w * S_W).astype(F8_NP).reshape(n_k2, 2, P, fo)
    return np.ascontiguousarray(a.transpose(2, 0, 1, 3))


def _w16(a):
    return np.ascontiguousarray(np.asarray(a, dtype=np.float32).astype(BF16_NP))


def _blocks_of(npad):
    """Split npad into <=512 near-equal multiples of 64, descending."""
    n_blk = (npad + 511) // 512
    base = (npad // n_blk) // 64 * 64
    blocks = [base] * n_blk
    rem = npad - base * n_blk
    i = 0
    while rem > 0:
        blocks[i] += 64
        rem -= 64
        i = (i + 1) % n_blk
    return sorted(blocks, reverse=True)


def _build_module(npad, blocks):
    nc = bacc.Bacc("TRN2", target_bir_lowering=False, debug=False)
    n_blk = len(blocks)
    offs = [sum(blocks[:i]) for i in range(n_blk)]
    bmax = max(blocks)

    dram = {}

    def din(name, shape, dt):
        dram[name] = nc.dram_tensor(name, list(shape), dt, kind="ExternalInput").ap()
        return dram[name]

    # x in DR layout per k2: [128, 2(grp), npad]
    for k2 in range(D_IN // 256):
        din(f"x_{k2}", (P, 2, npad), F8)
    din("epsT", (LAT, npad), F32)
    for name, (fi, fo) in ENC_LAYERS.items():
        din("w_" + name, (P, max(1, fi // 256), 2, fo), F8)
        din("b_" + name, (P if fo >= P else fo, max(1, fo // P)), F32)
    for name, (fi, fo) in DEC_LAYERS.items():
        din("w_" + name, (fi, fo), BF16)
        din("b_" + name, (P if fo >= P else fo, max(1, fo // P)), F32)
    outT = nc.dram_tensor("outT", [D_IN, npad], F32, kind="ExternalOutput").ap()

    with tile.TileContext(nc) as tc:
        with (
            tc.tile_pool(name="wpool", bufs=1) as wpool,
            tc.tile_pool(name="acts", bufs=2) as acts,
            tc.tile_pool(name="psum", bufs=6, space="PSUM") as psum,
        ):
            wsb = {}
            bsb = {}
            dma_rr = [0]

            def prologue_dma(out, in_):
                eng = nc.sync if dma_rr[0] % 2 == 0 else nc.scalar
                dma_rr[0] += 1
                eng.dma_start(out, in_)

            def load_enc_weights(name, prologue=False):
                if name in wsb:
                    return
                fi, fo = ENC_LAYERS[name]
                n_k2 = max(1, fi // 256)
                w_t = wpool.tile([P, n_k2, 2, fo], F8, tag=f"w_{name}", name=f"w_{name}")
                for k2 in range(n_k2):
                    for g in range(2):
                        src = dram["w_" + name][:, k2, g, :]
                        dst = w_t[:, k2, g, :]
                        (prologue_dma if prologue else nc.sync.dma_start)(dst, src)
                wsb[name] = w_t
                bp = P if fo >= P else fo
                b_t = wpool.tile([bp, max(1, fo // P)], F32, tag=f"b_{name}", name=f"b_{name}")
                nc.gpsimd.dma_start(b_t[:], dram["b_" + name][:])
                bsb[name] = b_t

            def load_dec_weights(name):
                if name in wsb:
                    return
                fi, fo = DEC_LAYERS[name]
                ktiles = []
                for k in range(max(1, fi // P)):
                    kp = min(P, fi)
                    w_t = wpool.tile([kp, fo], BF16, tag=f"w_{name}_{k}", name=f"w_{name}_{k}")
                    nc.gpsimd.dma_start(w_t[:], dram["w_" + name][k * P : k * P + kp, :])
                    ktiles.append(w_t)
                wsb[name] = ktiles
                bp = P if fo >= P else fo
                b_t = wpool.tile([bp, max(1, fo // P)], F32, tag=f"b_{name}", name=f"b_{name}")
                nc.gpsimd.dma_start(b_t[:], dram["b_" + name][:])
                bsb[name] = b_t

            # ---- activation tiles (all blocks resident per layer) ----
            x_in = [[None] * 2 for _ in range(n_blk)]      # [b][k2] fp8 [128,2,nb]
            h0 = [[None] * 4 for _ in range(n_blk)]
            h1 = [[None] * 2 for _ in range(n_blk)]
            h2 = [None] * n_blk
            mu_t = [None] * n_blk
            sg_t = [None] * n_blk
            z_t = [None] * n_blk
            h3 = [[None] * 2 for _ in range(n_blk)]
            h4 = [[None] * 4 for _ in range(n_blk)]
            h5 = [[None] * 8 for _ in range(n_blk)]

            eps_t = acts.tile([LAT, npad], F32, tag="eps", bufs=1, name="eps")
            nc.gpsimd.dma_start(eps_t[:], dram["epsT"][:])

            def load_x(b, prologue=False):
                nb, off = blocks[b], offs[b]
                for k2 in range(2):
                    t = acts.tile([P, 2, bmax], F8, tag=f"x_{k2}", bufs=n_blk, name=f"x_{k2}_{b}")
                    for g in range(2):
                        src = dram[f"x_{k2}"][:, g, off : off + nb]
                        (prologue_dma if prologue else nc.sync.dma_start)(t[:, g, :nb], src)
                    x_in[b][k2] = t

            def mm_fp8(name, b, ins, outs, funcs):
                """DR matmul layer: ins = list of [128,2,*] fp8 tiles (per k2).

                outs: list (per m) of (out_ap, func, scale, bias_ap) via funcs(m).
                """
                nb = blocks[b]
                fi, fo = ENC_LAYERS[name]
                w_t, b_t = wsb[name], bsb[name]
                n_k2 = max(1, fi // 256)
                n_m = max(1, fo // P)
                mp = min(P, fo)
                for m in range(n_m):
                    ps = psum.tile([P, 512], F32, tag="ps", name=f"ps_{name}_{m}_{b}")
                    for k2 in range(n_k2):
                        nc.tensor.matmul(
                            ps[:mp, :nb],
                            w_t[:, k2, :, m * mp : (m + 1) * mp],
                            ins[k2][:, :, :nb],
                            start=(k2 == 0),
                            stop=(k2 == n_k2 - 1),
                            perf_mode=DR,
                        )
                    funcs(m, ps, b_t)

            def mm_bf16(name, b, ins, evict):
                nb = blocks[b]
                fi, fo = DEC_LAYERS[name]
                wk, b_t = wsb[name], bsb[name]
                n_k = len(wk)
                n_m = max(1, fo // P)
                mp = min(P, fo)
                for m in range(n_m):
                    ps = psum.tile([P, 512], F32, tag="ps", name=f"ps_{name}_{m}_{b}")
                    for k in range(n_k):
                        nc.tensor.matmul(
                            ps[:mp, :nb],
                            wk[k][:, m * mp : (m + 1) * mp],
                            ins[k][:, :nb],
                            start=(k == 0),
                            stop=(k == n_k - 1),
                        )
                    evict(m, ps, b_t)

            # ---- per-layer stage emitters ----
            def enc0_stage(b):
                nb = blocks[b]

                def ev(m, ps, b_t):
                    t = h0[b][m // 2]
                    if t is None:
                        t = acts.tile([P, 2, bmax], F8, tag=f"h0_{m // 2}", bufs=n_blk, name=f"h0_{m // 2}_{b}")
                        h0[b][m // 2] = t
                    nc.scalar.activation(
                        t[:, m % 2, :nb], ps[:, :nb], AF.Relu,
                        bias=b_t[:, m : m + 1], scale=S_A / (S_W * S_X),
                    )

                mm_fp8("enc0", b, x_in[b], None, ev)

            def encu_stage(b):
                nb = blocks[b]

                def ev(m, ps, b_t):
                    t = h1[b][m // 2]
                    if t is None:
                        t = acts.tile([P, 2, bmax], F8, tag=f"h1_{m // 2}", bufs=n_blk, name=f"h1_{m // 2}_{b}")
                        h1[b][m // 2] = t
                    nc.scalar.activation(
                        t[:, m % 2, :nb], ps[:, :nb], AF.Relu,
                        bias=b_t[:, m : m + 1], scale=1.0 / S_W,
                    )

                mm_fp8("encu", b, h0[b], None, ev)

            def enc2_stage(b):
                nb = blocks[b]

                def ev(m, ps, b_t):
                    t = h2[b]
                    if t is None:
                        t = acts.tile([P, 2, bmax], F8, tag="h2", bufs=n_blk, name=f"h2_{b}")
                        h2[b] = t
                    nc.scalar.activation(
                        t[:, m, :nb], ps[:, :nb], AF.Relu,
                        bias=b_t[:, m : m + 1], scale=1.0 / S_W,
                    )

                mm_fp8("enc2", b, h1[b], None, ev)

            def mulv_stage(b):
                nb = blocks[b]

                def ev(m, ps, b_t):
                    mu = acts.tile([LAT, bmax], F32, tag="mu", bufs=2, name=f"mu_{b}")
                    sg = acts.tile([LAT, bmax], F32, tag="sg", bufs=2, name=f"sg_{b}")
                    nc.scalar.activation(
                        mu[:, :nb], ps[:LAT, :nb], AF.Identity,
                        bias=b_t[:LAT, 0:1], scale=1.0 / (S_A * S_W),
                    )
                    nc.scalar.activation(
                        sg[:, :nb], ps[LAT:, :nb], AF.Exp,
                        bias=b_t[LAT:, 0:1], scale=0.5 / (S_A * S_W),
                    )
                    mu_t[b], sg_t[b] = mu, sg

                mm_fp8("mulv", b, [h2[b]], None, ev)

            def lat_stage(b):
                nb, off = blocks[b], offs[b]
                tmp = acts.tile([LAT, bmax], F32, tag="tmp", bufs=2, name=f"tmp_{b}")
                nc.vector.tensor_mul(tmp[:, :nb], sg_t[b][:, :nb], eps_t[:, off : off + nb])
                z = acts.tile([LAT, bmax], BF16, tag="z", bufs=n_blk, name=f"z_{b}")
                nc.vector.tensor_add(z[:, :nb], tmp[:, :nb], mu_t[b][:, :nb])
                z_t[b] = z

            def dec_stage(name, b, ins, hout, n_out, relu=True):
                nb = blocks[b]

                def ev(m, ps, b_t):
                    t = acts.tile([P, bmax], BF16, tag=f"{name}_{m}", bufs=n_blk, name=f"{name}_{m}_{b}")
                    nc.vector.tensor_scalar(
                        t[:, :nb], ps[:, :nb], b_t[:, m : m + 1], 0.0, ALU.add, ALU.max
                    )
                    hout[b][m] = t

                mm_bf16(name, b, ins, ev)

            def fin_stage(b):
                nb, off = blocks[b], offs[b]

                def ev(m, ps, b_t):
                    t = acts.tile([P, bmax], F32, tag=f"o_{m}", bufs=3, name=f"o_{m}_{b}")
                    nc.vector.tensor_scalar(
                        t[:, :nb], ps[:, :nb], b_t[:, m : m + 1], None, ALU.add
                    )
                    nc.sync.dma_start(outT[m * P : (m + 1) * P, off : off + nb], t[:, :nb])

                mm_bf16("fin", b, h5[b], ev)

            # ---- warm up the PE while prologue DMAs stream in ----
            wu_w = wpool.tile([P, P], BF16, tag="wu_w", name="wu_w")
            wu_x = wpool.tile([P, P], BF16, tag="wu_x", name="wu_x")
            nc.vector.memset(wu_w[:], 0.0)
            nc.vector.memset(wu_x[:], 0.0)
            wu_ps = psum.tile([P, 512], F32, tag="wu_ps", bufs=1, name="wu_ps")
            for _ in range(10):
                nc.tensor.matmul(wu_ps[:, :P], wu_w[:], wu_x[:], start=True, stop=True)

            # ---- prologue DMAs ----
            load_enc_weights("enc0", prologue=True)
            load_x(0, prologue=True)
            for b in range(1, n_blk):
                load_x(b)
            # decoder weights + eps + biases stream on gpsimd from the start
            for name in DEC_LAYERS:
                load_dec_weights(name)

            # ---- layer wavefront ----
            for b in range(n_blk):
                enc0_stage(b)
            load_enc_weights("encu")
            for b in range(n_blk):
                encu_stage(b)
            load_enc_weights("enc2")
            load_enc_weights("mulv")
            # stagger: mulv/latent of block b hides under enc2 of blocks b+2..
            enc2_stage(0)
            if n_blk > 1:
                enc2_stage(1)
            for b in range(n_blk):
                if b + 2 < n_blk:
                    enc2_stage(b + 2)
                mulv_stage(b)
                lat_stage(b)
            for b in range(n_blk):
                dec_stage("dec0", b, [z_t[b]], h3, 2)
            for b in range(n_blk):
                dec_stage("dec1", b, h3[b], h4, 4)
            for b in range(n_blk):
                dec_stage("dec2", b, h4[b], h5, 8)
            for b in range(n_blk):
                fin_stage(b)

    nc.compile()
    return nc


def kernel(**inputs):
    x = np.asarray(inputs["x"], dtype=np.float32)
    lbl = np.asarray(inputs["cluster_labels"]).astype(np.int64)
    eps = np.asarray(inputs["eps"], dtype=np.float32)
    B = x.shape[0]

    counts = np.bincount(lbl, minlength=C)
    npad = max(512, _ceil_to(int(counts.max()), 64))
    blocks = _blocks_of(npad)

    rows = [np.nonzero(lbl == c)[0] for c in range(C)]

    mulv_W = np.concatenate([np.asarray(inputs["mu_W"]), np.asarray(inputs["lv_W"])], axis=1)
    mulv_b = np.concatenate([np.asarray(inputs["mu_b"]), 0.5 * np.asarray(inputs["lv_b"])])

    shared = {
        "w_enc0": _w8(inputs["enc_W0"]),
        "b_enc0": _b2d(S_A * np.asarray(inputs["enc_b0"])),
        "w_enc2": _w8(inputs["enc_W2"]),
        "b_enc2": _b2d(S_A * np.asarray(inputs["enc_b2"])),
        "w_mulv": _w8(mulv_W),
        "b_mulv": _b2d(mulv_b),
        "w_dec1": _w16(inputs["dec_W1"]),
        "b_dec1": _b2d(inputs["dec_b1"]),
    }

    in_maps = []
    for c in range(C):
        r = rows[c]
        xT = np.zeros((D_IN, npad), np.float32)
        xT[:, : len(r)] = x[r].T
        x_dr = (xT * S_X).astype(F8_NP).reshape(2, 2, P, npad).transpose(2, 0, 1, 3)
        epsT = np.zeros((LAT, npad), np.float32)
        epsT[:, : len(r)] = eps[r].T
        m = dict(shared)
        m["x_0"] = np.ascontiguousarray(x_dr[:, 0])
        m["x_1"] = np.ascontiguousarray(x_dr[:, 1])
        m["epsT"] = epsT
        m["w_encu"] = _w8(inputs["enc_Wu"][c])
        m["b_encu"] = _b2d(S_A * np.asarray(inputs["enc_bu"][c]))
        m["w_dec0"] = _w16(inputs["dec_Wu0"][c])
        m["b_dec0"] = _b2d(inputs["dec_bu0"][c])
        m["w_dec2"] = _w16(inputs["dec_Wu2"][c])
        m["b_dec2"] = _b2d(inputs["dec_bu2"][c])
        m["w_fin"] = _w16(inputs["fin_W"][c])
        m["b_fin"] = _b2d(inputs["fin_b"][c])
        in_maps.append(m)

    nc = _build_module(npad, blocks)
    res = bass_utils.run_bass_kernel_spmd(nc, in_maps, core_ids=list(range(N_CORES)))
    global LAST_RESULTS
    LAST_RESULTS = res

    out = np.empty((B, D_IN), np.float32)
    for c in range(C):
        r = rows[c]
        out[r] = res.results[c]["outT"][:, : len(r)].T
    return out


# revision 5
# speedup vs baseline: 1.2229x; 1.0390x over previous
"""CISS-VAE (per-cluster MoE-routed MLP chain) Trainium2 kernel.

Strategy (routing on host, compute on device):
  - Rows are grouped by cluster label on the host; core c processes all rows
    of cluster c (C == n_cores == 8), so every GEMM is a dense per-cluster
    GEMM (no 8x redundant einsum like the reference).
  - The encoder (enc0, encu, enc2, fused mu|lv head) runs in fp8-e4m3 with
    DoubleRow matmuls (2 fp8 k-rows per cell): the VAE latent z is dominated
    by the eps noise term, so encoder-side quantization error is attenuated
    far below the decoder's sensitivity (measured end-to-end rel err 1.9e-3
    vs 1.8e-3 all-bf16). The decoder stays bf16 (decoder-side fp8 measured
    1.2e-2..3.3e-2, too close to the 2e-2 gate).
  - fp8 scales chosen so every encoder PSUM eviction is scale-free
    (sigma_out == sigma_w * sigma_in): x*4, enc0 weights *16, hidden
    activations *64, encu/enc2 weights *1. Scale-free evictions are a
    2-ALU-op pattern (add bias, max 0) so they can run on the Vector and
    GpSimd engines, not just Scalar - the eviction work for thin layers is
    spread over three engines so the PE never throttles on PSUM drain.
  - Schedule is a layer-wavefront: each layer runs over all row blocks
    before the next starts, so a block's evictions overlap other blocks'
    matmuls. The thin middle layers (enc2, mu|lv head, latent chain, dec0,
    dec1) are software-staggered into one interleaved stream so the
    mu/sigma/z chain of block b hides under the matmuls of neighboring
    blocks. npad splits into near-equal blocks (<=512 for one PSUM bank),
    largest first so the tail block is smallest.
  - Prologue DMAs are fine-grained (x and first-layer weights split in
    ~32-114KB pieces across the sync+scalar queues) so the first real
    matmul issues ~0.5us in; a short dummy-matmul warmup covers the rest.
"""

import ml_dtypes
import numpy as np

import concourse.bacc as bacc
import concourse.mybir as mybir
import concourse.tile as tile
from concourse import bass_utils

P = 128
D_IN, LAT, C = 512, 64, 8
H0, H1, H2 = 1024, 512, 256
N_CORES = 8
F32 = mybir.dt.float32
BF16 = mybir.dt.bfloat16
F8 = mybir.dt.float8e4
AF = mybir.ActivationFunctionType
ALU = mybir.AluOpType
DR = mybir.MatmulPerfMode.DoubleRow
BF16_NP = ml_dtypes.bfloat16
F8_NP = ml_dtypes.float8_e4m3

S_X = 4.0    # fp8 scale on the x input
S_W0 = 16.0  # fp8 scale on enc0 weights
S_H = 64.0   # fp8 scale on encoder hidden activations (== S_X*S_W0 == 1*S_H)
S_WM = 16.0  # fp8 scale on the mu|lv head weights

ENC_LAYERS = dict(enc0=(D_IN, H0), encu=(H0, H1), enc2=(H1, H2), mulv=(H2, 2 * LAT))
DEC_LAYERS = dict(dec0=(LAT, H2), dec1=(H2, H1), dec2=(H1, H0), fin=(H0, D_IN))


def _ceil_to(x, m):
    return ((x + m - 1) // m) * m


def _b2d(b):
    b = np.asarray(b, np.float32)
    f = b.shape[0]
    if f >= P:
        return np.ascontiguousarray(b.reshape(f // P, P).T)
    return np.ascontiguousarray(b.reshape(1, f).T)


def _w8(w, scale):
    """[fi, fo] weight -> fp8 DR layout [128, n_k2, 2, fo]."""
    w = np.asarray(w, np.float32)
    fi, fo = w.shape
    n_k2 = max(1, fi // 256)
    a = (w * scale).astype(F8_NP).reshape(n_k2, 2, P, fo)
    return np.ascontiguousarray(a.transpose(2, 0, 1, 3))


def _w16(a):
    return np.ascontiguousarray(np.asarray(a, dtype=np.float32).astype(BF16_NP))


def _blocks_of(npad):
    """Split npad into <=512 near-equal multiples of 64, descending."""
    n_blk = (npad + 511) // 512
    base = (npad // n_blk) // 64 * 64
    blocks = [base] * n_blk
    rem = npad - base * n_blk
    i = 0
    while rem > 0:
        blocks[i] += 64
        rem -= 64
        i = (i + 1) % n_blk
    return sorted(blocks, reverse=True)


def _build_module(npad, blocks):
    nc = bacc.Bacc("TRN2", target_bir_lowering=False, debug=False)
    n_blk = len(blocks)
    offs = [sum(blocks[:i]) for i in range(n_blk)]
    bmax = max(blocks)

    dram = {}

    def din(name, shape, dt):
        dram[name] = nc.dram_tensor(name, list(shape), dt, kind="ExternalInput").ap()
        return dram[name]

    for k2 in range(D_IN // 256):
        din(f"x_{k2}", (P, 2, npad), F8)
    din("epsT", (LAT, npad), F32)
    for name, (fi, fo) in ENC_LAYERS.items():
        din("w_" + name, (P, max(1, fi // 256), 2, fo), F8)
        din("b_" + name, (P if fo >= P else fo, max(1, fo // P)), F32)
    for name, (fi, fo) in DEC_LAYERS.items():
        din("w_" + name, (fi, fo), BF16)
        din("b_" + name, (P if fo >= P else fo, max(1, fo // P)), F32)
    outT = nc.dram_tensor("outT", [D_IN, npad], F32, kind="ExternalOutput").ap()

    with tile.TileContext(nc) as tc:
        with (
            tc.tile_pool(name="wpool", bufs=1) as wpool,
            tc.tile_pool(name="acts", bufs=2) as acts,
            tc.tile_pool(name="psum", bufs=6, space="PSUM") as psum,
        ):
            wsb = {}
            bsb = {}

            def load_bias(name, table):
                fi, fo = table[name]
                bp = P if fo >= P else fo
                b_t = wpool.tile([bp, max(1, fo // P)], F32, tag=f"b_{name}", name=f"b_{name}")
                nc.gpsimd.dma_start(b_t[:], dram["b_" + name][:])
                bsb[name] = b_t

            def load_enc_weights(name):
                if name in wsb:
                    return
                fi, fo = ENC_LAYERS[name]
                n_k2 = max(1, fi // 256)
                w_t = wpool.tile([P, n_k2, 2, fo], F8, tag=f"w_{name}", name=f"w_{name}")
                for k2 in range(n_k2):
                    for g in range(2):
                        nc.sync.dma_start(w_t[:, k2, g, :], dram["w_" + name][:, k2, g, :])
                wsb[name] = w_t
                load_bias(name, ENC_LAYERS)

            def load_dec_weights(name):
                if name in wsb:
                    return
                fi, fo = DEC_LAYERS[name]
                ktiles = []
                for k in range(max(1, fi // P)):
                    kp = min(P, fi)
                    w_t = wpool.tile([kp, fo], BF16, tag=f"w_{name}_{k}", name=f"w_{name}_{k}")
                    nc.gpsimd.dma_start(w_t[:], dram["w_" + name][k * P : k * P + kp, :])
                    ktiles.append(w_t)
                wsb[name] = ktiles
                load_bias(name, DEC_LAYERS)

            # ---- activation tiles ----
            x_in = [[None] * 2 for _ in range(n_blk)]
            h0 = [[None] * 4 for _ in range(n_blk)]
            h1 = [[None] * 2 for _ in range(n_blk)]
            h2 = [None] * n_blk
            mu_t = [None] * n_blk
            sg_t = [None] * n_blk
            z_t = [None] * n_blk
            h3 = [[None] * 2 for _ in range(n_blk)]
            h4 = [[None] * 4 for _ in range(n_blk)]
            h5 = [[None] * 8 for _ in range(n_blk)]

            eps_t = acts.tile([LAT, npad], F32, tag="eps", bufs=1, name="eps")

            def load_x(b):
                nb, off = blocks[b], offs[b]
                for k2 in range(2):
                    t = acts.tile([P, 2, bmax], F8, tag=f"x_{k2}", bufs=n_blk, name=f"x_{k2}_{b}")
                    for g in range(2):
                        nc.sync.dma_start(t[:, g, :nb], dram[f"x_{k2}"][:, g, off : off + nb])
                    x_in[b][k2] = t

            def mm_fp8(name, b, ins, evict):
                nb = blocks[b]
                fi, fo = ENC_LAYERS[name]
                w_t, b_t = wsb[name], bsb[name]
                n_k2 = max(1, fi // 256)
                n_m = max(1, fo // P)
                mp = min(P, fo)
                for m in range(n_m):
                    ps = psum.tile([P, 512], F32, tag="ps", name=f"ps_{name}_{m}_{b}")
                    for k2 in range(n_k2):
                        nc.tensor.matmul(
                            ps[:mp, :nb],
                            w_t[:, k2, :, m * mp : (m + 1) * mp],
                            ins[k2][:, :, :nb],
                            start=(k2 == 0),
                            stop=(k2 == n_k2 - 1),
                            perf_mode=DR,
                        )
                    evict(m, ps, b_t)

            def mm_bf16(name, b, ins, evict):
                nb = blocks[b]
                fi, fo = DEC_LAYERS[name]
                wk, b_t = wsb[name], bsb[name]
                n_k = len(wk)
                n_m = max(1, fo // P)
                mp = min(P, fo)
                for m in range(n_m):
                    ps = psum.tile([P, 512], F32, tag="ps", name=f"ps_{name}_{m}_{b}")
                    for k in range(n_k):
                        nc.tensor.matmul(
                            ps[:mp, :nb],
                            wk[k][:, m * mp : (m + 1) * mp],
                            ins[k][:, :nb],
                            start=(k == 0),
                            stop=(k == n_k - 1),
                        )
                    evict(m, ps, b_t)

            def relu_evict(eng, out_ap, ps_ap, bias_ap):
                """out = relu(ps + bias); scale-free, runs on any engine."""
                if eng is nc.scalar:
                    nc.scalar.activation(out_ap, ps_ap, AF.Relu, bias=bias_ap, scale=1.0)
                else:
                    eng.tensor_scalar(out_ap, ps_ap, bias_ap, 0.0, ALU.add, ALU.max)

            # ---- per-layer stages ----
            def enc0_stage(b):
                nb = blocks[b]

                def ev(m, ps, b_t):
                    t = h0[b][m // 2]
                    if t is None:
                        t = acts.tile([P, 2, bmax], F8, tag=f"h0_{m // 2}", bufs=n_blk, name=f"h0_{m // 2}_{b}")
                        h0[b][m // 2] = t
                    eng = nc.scalar if m % 2 == 0 else nc.vector
                    relu_evict(eng, t[:, m % 2, :nb], ps[:, :nb], b_t[:, m : m + 1])

                mm_fp8("enc0", b, x_in[b], ev)

            def encu_stage(b):
                nb = blocks[b]

                def ev(m, ps, b_t):
                    t = h1[b][m // 2]
                    if t is None:
                        t = acts.tile([P, 2, bmax], F8, tag=f"h1_{m // 2}", bufs=n_blk, name=f"h1_{m // 2}_{b}")
                        h1[b][m // 2] = t
                    eng = nc.scalar if m % 2 == 0 else nc.vector
                    relu_evict(eng, t[:, m % 2, :nb], ps[:, :nb], b_t[:, m : m + 1])

                mm_fp8("encu", b, h0[b], ev)

            def enc2_stage(b):
                nb = blocks[b]

                def ev(m, ps, b_t):
                    t = h2[b]
                    if t is None:
                        t = acts.tile([P, 2, bmax], F8, tag="h2", bufs=n_blk, name=f"h2_{b}")
                        h2[b] = t
                    eng = nc.scalar if m % 2 == 0 else nc.vector
                    relu_evict(eng, t[:, m, :nb], ps[:, :nb], b_t[:, m : m + 1])

                mm_fp8("enc2", b, h1[b], ev)

            def mulv_stage(b):
                nb = blocks[b]

                def ev(m, ps, b_t):
                    mu = acts.tile([LAT, bmax], F32, tag="mu", bufs=2, name=f"mu_{b}")
                    sg = acts.tile([LAT, bmax], F32, tag="sg", bufs=2, name=f"sg_{b}")
                    nc.scalar.activation(
                        mu[:, :nb], ps[:LAT, :nb], AF.Identity,
                        bias=b_t[:LAT, 0:1], scale=1.0 / (S_H * S_WM),
                    )
                    nc.scalar.activation(
                        sg[:, :nb], ps[LAT:, :nb], AF.Exp,
                        bias=b_t[LAT:, 0:1], scale=0.5 / (S_H * S_WM),
                    )
                    mu_t[b], sg_t[b] = mu, sg

                mm_fp8("mulv", b, [h2[b]], ev)

            def lat_stage(b):
                nb, off = blocks[b], offs[b]
                tmp = acts.tile([LAT, bmax], F32, tag="tmp", bufs=2, name=f"tmp_{b}")
                nc.vector.tensor_mul(tmp[:, :nb], sg_t[b][:, :nb], eps_t[:, off : off + nb])
                z = acts.tile([LAT, bmax], BF16, tag="z", bufs=n_blk, name=f"z_{b}")
                nc.vector.tensor_add(z[:, :nb], tmp[:, :nb], mu_t[b][:, :nb])
                z_t[b] = z

            def dec0_stage(b):
                nb = blocks[b]

                def ev(m, ps, b_t):
                    t = acts.tile([P, bmax], BF16, tag=f"h3_{m}", bufs=n_blk, name=f"h3_{m}_{b}")
                    eng = nc.scalar if m % 2 == 0 else nc.vector
                    relu_evict(eng, t[:, :nb], ps[:, :nb], b_t[:, m : m + 1])
                    h3[b][m] = t

                mm_bf16("dec0", b, [z_t[b]], ev)

            def dec1_stage(b):
                nb = blocks[b]

                def ev(m, ps, b_t):
                    t = acts.tile([P, bmax], BF16, tag=f"h4_{m}", bufs=n_blk, name=f"h4_{m}_{b}")
                    eng = nc.scalar if m % 2 == 0 else nc.vector
                    relu_evict(eng, t[:, :nb], ps[:, :nb], b_t[:, m : m + 1])
                    h4[b][m] = t

                mm_bf16("dec1", b, h3[b], ev)

            def dec2_stage(b):
                nb = blocks[b]

                def ev(m, ps, b_t):
                    t = acts.tile([P, bmax], BF16, tag=f"h5_{m}", bufs=n_blk, name=f"h5_{m}_{b}")
                    eng = nc.scalar if m % 2 == 0 else nc.vector
                    relu_evict(eng, t[:, :nb], ps[:, :nb], b_t[:, m : m + 1])
                    h5[b][m] = t

                mm_bf16("dec2", b, h4[b], ev)

            def fin_stage(b):
                nb, off = blocks[b], offs[b]

                def ev(m, ps, b_t):
                    t = acts.tile([P, bmax], F32, tag=f"o_{m}", bufs=3, name=f"o_{m}_{b}")
                    if m % 2 == 0:
                        nc.scalar.activation(t[:, :nb], ps[:, :nb], AF.Identity, bias=b_t[:, m : m + 1], scale=1.0)
                    else:
                        nc.vector.tensor_scalar(t[:, :nb], ps[:, :nb], b_t[:, m : m + 1], None, ALU.add)
                    nc.sync.dma_start(outT[m * P : (m + 1) * P, off : off + nb], t[:, :nb])

                mm_bf16("fin", b, h5[b], ev)

            # ---- warm up the PE while prologue DMAs stream in ----
            wu_w = wpool.tile([P, P], BF16, tag="wu_w", name="wu_w")
            wu_x = wpool.tile([P, P], BF16, tag="wu_x", name="wu_x")
            nc.vector.memset(wu_w[:], 0.0)
            nc.vector.memset(wu_x[:], 0.0)
            wu_ps = psum.tile([P, 512], F32, tag="wu_ps", bufs=1, name="wu_ps")
            for _ in range(8):
                nc.tensor.matmul(wu_ps[:, :P], wu_w[:], wu_x[:], start=True, stop=True)

            # ---- fine-grained prologue: first-MM dependencies first ----
            # enc0 weights [128, k2, g, 1024] split into 256-col quarters;
            # x(b0) slabs split per (k2, g). sync gets g=0, scalar gets g=1.
            w0_t = wpool.tile([P, 2, 2, H0], F8, tag="w_enc0", name="w_enc0")
            wsb["enc0"] = w0_t
            nb0 = blocks[0]
            x00 = acts.tile([P, 2, bmax], F8, tag="x_0", bufs=n_blk, name="x_0_0")
            x01 = acts.tile([P, 2, bmax], F8, tag="x_1", bufs=n_blk, name="x_1_0")
            x_in[0] = [x00, x01]
            nc.sync.dma_start(x00[:, 0, :nb0], dram["x_0"][:, 0, :nb0])
            nc.scalar.dma_start(x00[:, 1, :nb0], dram["x_0"][:, 1, :nb0])
            nc.sync.dma_start(w0_t[:, 0, 0, 0:256], dram["w_enc0"][:, 0, 0, 0:256])
            nc.scalar.dma_start(w0_t[:, 0, 1, 0:256], dram["w_enc0"][:, 0, 1, 0:256])
            nc.sync.dma_start(x01[:, 0, :nb0], dram["x_1"][:, 0, :nb0])
            nc.scalar.dma_start(x01[:, 1, :nb0], dram["x_1"][:, 1, :nb0])
            nc.sync.dma_start(w0_t[:, 1, 0, 0:256], dram["w_enc0"][:, 1, 0, 0:256])
            nc.scalar.dma_start(w0_t[:, 1, 1, 0:256], dram["w_enc0"][:, 1, 1, 0:256])
            for q in range(1, 4):
                for k2 in range(2):
                    nc.sync.dma_start(
                        w0_t[:, k2, 0, q * 256 : (q + 1) * 256],
                        dram["w_enc0"][:, k2, 0, q * 256 : (q + 1) * 256],
                    )
                    nc.scalar.dma_start(
                        w0_t[:, k2, 1, q * 256 : (q + 1) * 256],
                        dram["w_enc0"][:, k2, 1, q * 256 : (q + 1) * 256],
                    )
            load_bias("enc0", ENC_LAYERS)
            for b in range(1, n_blk):
                load_x(b)
            nc.gpsimd.dma_start(eps_t[:], dram["epsT"][:])
            for name in DEC_LAYERS:
                load_dec_weights(name)

            # ---- layer wavefront ----
            for b in range(n_blk):
                enc0_stage(b)
            load_enc_weights("encu")
            for b in range(n_blk):
                encu_stage(b)
            load_enc_weights("enc2")
            load_enc_weights("mulv")
            # Full staggered wavefront for the remaining 6 layers + latent
            # chain: block b's thin stages (enc2, mu|lv, z) interleave with
            # neighboring blocks' thick decoder matmuls, so the PSUM
            # eviction engines always have 2x slack vs the PE stream and
            # the z-chain latency hides under a full iteration of matmuls.
            def maybe(stage, b):
                if 0 <= b < n_blk:
                    stage(b)

            for i in range(n_blk + 5):
                maybe(enc2_stage, i)
                maybe(mulv_stage, i - 1)
                maybe(lat_stage, i - 1)
                maybe(dec0_stage, i - 2)
                maybe(dec1_stage, i - 3)
                maybe(dec2_stage, i - 4)
                maybe(fin_stage, i - 5)

    nc.compile()
    return nc


def kernel(**inputs):
    x = np.asarray(inputs["x"], dtype=np.float32)
    lbl = np.asarray(inputs["cluster_labels"]).astype(np.int64)
    eps = np.asarray(inputs["eps"], dtype=np.float32)
    B = x.shape[0]

    counts = np.bincount(lbl, minlength=C)
    npad = max(512, _ceil_to(int(counts.max()), 64))
    blocks = _blocks_of(npad)

    rows = [np.nonzero(lbl == c)[0] for c in range(C)]

    mulv_W = np.concatenate([np.asarray(inputs["mu_W"]), np.asarray(inputs["lv_W"])], axis=1)
    mulv_b = np.concatenate([np.asarray(inputs["mu_b"]), 0.5 * np.asarray(inputs["lv_b"])])

    shared = {
        "w_enc0": _w8(inputs["enc_W0"], S_W0),
        "b_enc0": _b2d(S_H * np.asarray(inputs["enc_b0"])),
        "w_enc2": _w8(inputs["enc_W2"], 1.0),
        "b_enc2": _b2d(S_H * np.asarray(inputs["enc_b2"])),
        "w_mulv": _w8(mulv_W, S_WM),
        "b_mulv": _b2d(mulv_b),
        "w_dec1": _w16(inputs["dec_W1"]),
        "b_dec1": _b2d(inputs["dec_b1"]),
    }

    in_maps = []
    for c in range(C):
        r = rows[c]
        xT = np.zeros((D_IN, npad), np.float32)
        xT[:, : len(r)] = x[r].T
        x_dr = (xT * S_X).astype(F8_NP).reshape(2, 2, P, npad).transpose(2, 0, 1, 3)
        epsT = np.zeros((LAT, npad), np.float32)
        epsT[:, : len(r)] = eps[r].T
        m = dict(shared)
        m["x_0"] = np.ascontiguousarray(x_dr[:, 0])
        m["x_1"] = np.ascontiguousarray(x_dr[:, 1])
        m["epsT"] = epsT
        m["w_encu"] = _w8(inputs["enc_Wu"][c], 1.0)
        m["b_encu"] = _b2d(S_H * np.asarray(inputs["enc_bu"][c]))
        m["w_dec0"] = _w16(inputs["dec_Wu0"][c])
        m["b_dec0"] = _b2d(inputs["dec_bu0"][c])
        m["w_dec2"] = _w16(inputs["dec_Wu2"][c])
        m["b_dec2"] = _b2d(inputs["dec_bu2"][c])
        m["w_fin"] = _w16(inputs["fin_W"][c])
        m["b_fin"] = _b2d(inputs["fin_b"][c])
        in_maps.append(m)

    nc = _build_module(npad, blocks)
    res = bass_utils.run_bass_kernel_spmd(nc, in_maps, core_ids=list(range(N_CORES)))
    global LAST_RESULTS
    LAST_RESULTS = res

    out = np.empty((B, D_IN), np.float32)
    for c in range(C):
        r = rows[c]
        out[r] = res.results[c]["outT"][:, : len(r)].T
    return out


# revision 9
# speedup vs baseline: 1.2881x; 1.0533x over previous
"""CISS-VAE (per-cluster MoE-routed MLP chain) Trainium2 kernel.

Strategy (routing on host, compute on device):
  - Rows are grouped by cluster label on the host; core c processes all rows
    of cluster c (C == n_cores == 8), so every GEMM is a dense per-cluster
    GEMM (no 8x redundant einsum like the reference).
  - The encoder (enc0, encu, enc2, fused mu|lv head) runs in fp8-e4m3 with
    DoubleRow matmuls (2 fp8 k-rows per cell, ~2x bf16 rate): the VAE latent
    z is dominated by the eps noise term, so encoder-side quantization error
    is attenuated far below the decoder's sensitivity (measured end-to-end
    rel err 1.9e-3 vs 1.8e-3 all-bf16). The decoder stays bf16 (decoder-side
    fp8 measured 1.2e-2..3.3e-2, too close to the 2e-2 gate).
  - fp8 scales chosen so every encoder PSUM eviction is scale-free
    (sigma_out == sigma_w * sigma_in): x*4, enc0 weights *16, hidden
    activations *64, encu/enc2 weights *1. Scale-free evictions are a
    2-ALU-op pattern (add bias, max 0) so they split across the Scalar AND
    Vector engines - eviction throughput, not the PE, limits the thin
    layers, so both engines share the drain work everywhere.
  - Schedule: enc0 runs as a plain layer-wavefront over the row blocks,
    then encu/enc2/mu|lv/latent/dec0/dec1/dec2/fin advance as one staggered
    software pipeline (layer L of block b interleaves with neighboring
    blocks' other layers), so thin stages always have thick matmuls in
    flight around them and the mu/sigma/z chain latency is hidden. npad is
    split into near-equal blocks (<=512 = one PSUM bank), largest first.
  - DMA descriptor issue costs ~0.65us of engine time each, so transfers
    are merged: one DMA per x block, per-k2 weight slabs, one merged
    [128, n_k*fo] tile per decoder layer, one bias blob, one output DMA per
    block. Issues spread across the sync/scalar/vector/gpsimd queues ahead
    of the eviction streams.
"""

import ml_dtypes
import numpy as np

import concourse.bacc as bacc
import concourse.mybir as mybir
import concourse.tile as tile
from concourse import bass_utils

P = 128
D_IN, LAT, C = 512, 64, 8
H0, H1, H2 = 1024, 512, 256
N_CORES = 8
F32 = mybir.dt.float32
BF16 = mybir.dt.bfloat16
F8 = mybir.dt.float8e4
AF = mybir.ActivationFunctionType
ALU = mybir.AluOpType
DR = mybir.MatmulPerfMode.DoubleRow
BF16_NP = ml_dtypes.bfloat16
F8_NP = ml_dtypes.float8_e4m3

S_X = 4.0    # fp8 scale on the x input
S_W0 = 16.0  # fp8 scale on enc0 weights
S_H = 64.0   # fp8 scale on encoder hidden activations (== S_X*S_W0 == 1*S_H)
S_WM = 16.0  # fp8 scale on the mu|lv head weights

ENC_LAYERS = dict(enc0=(D_IN, H0), encu=(H0, H1), enc2=(H1, H2), mulv=(H2, 2 * LAT))
DEC_LAYERS = dict(dec0=(LAT, H2), dec1=(H2, H1), dec2=(H1, H0), fin=(H0, D_IN))
BIAS_ORDER = ["enc0", "encu", "enc2", "mulv", "dec0", "dec1", "dec2", "fin"]


def _ceil_to(x, m):
    return ((x + m - 1) // m) * m


def _bias_cols(name):
    table = ENC_LAYERS if name in ENC_LAYERS else DEC_LAYERS
    return max(1, table[name][1] // P)


def _w8(w, scale):
    """[fi, fo] weight -> fp8 DR layout [128, n_k2, 2, fo]."""
    w = np.asarray(w, np.float32)
    fi, fo = w.shape
    n_k2 = max(1, fi // 256)
    a = (w * scale).astype(F8_NP).reshape(n_k2, 2, P, fo)
    return np.ascontiguousarray(a.transpose(2, 0, 1, 3))


def _wdec(w):
    """[fi, fo] bf16 weight -> merged [kp, n_k*fo] (k-slabs side by side)."""
    w = np.asarray(w, np.float32).astype(BF16_NP)
    fi, fo = w.shape
    if fi <= P:
        return np.ascontiguousarray(w)
    n_k = fi // P
    return np.ascontiguousarray(w.reshape(n_k, P, fo).transpose(1, 0, 2).reshape(P, n_k * fo))


def _blocks_of(npad):
    """Split npad into <=512 near-equal multiples of 64, descending."""
    n_blk = (npad + 511) // 512
    base = (npad // n_blk) // 64 * 64
    blocks = [base] * n_blk
    rem = npad - base * n_blk
    i = 0
    while rem > 0:
        blocks[i] += 64
        rem -= 64
        i = (i + 1) % n_blk
    return sorted(blocks, reverse=True)


def _build_module(npad, blocks):
    nc = bacc.Bacc("TRN2", target_bir_lowering=False, debug=False)
    n_blk = len(blocks)
    offs = [sum(blocks[:i]) for i in range(n_blk)]
    bmax = max(blocks)

    dram = {}

    def din(name, shape, dt):
        dram[name] = nc.dram_tensor(name, list(shape), dt, kind="ExternalInput").ap()
        return dram[name]

    din("x_dr", (P, 4, npad), F8)  # dim1 = k2*2 + grp
    din("epsT", (LAT, npad), F32)
    for name, (fi, fo) in ENC_LAYERS.items():
        din("w_" + name, (P, max(1, fi // 256), 2, fo), F8)
    for name, (fi, fo) in DEC_LAYERS.items():
        kp = min(P, fi)
        din("w_" + name, (kp, max(1, fi // P) * fo), BF16)
    n_bias = sum(_bias_cols(n) for n in BIAS_ORDER)
    din("biases", (P, n_bias), F32)
    outT = nc.dram_tensor("outT", [P, 4, npad], F32, kind="ExternalOutput").ap()

    with tile.TileContext(nc) as tc:
        with (
            tc.tile_pool(name="wpool", bufs=1) as wpool,
            tc.tile_pool(name="acts", bufs=2) as acts,
            tc.tile_pool(name="psum", bufs=8, space="PSUM") as psum,
        ):
            wsb = {}

            # ---- warm up the PE while engines/queues boot ----
            wu_w = wpool.tile([P, P], BF16, tag="wu_w", name="wu_w")
            wu_x = wpool.tile([P, P], BF16, tag="wu_x", name="wu_x")
            nc.vector.memset(wu_w[:], 0.0)
            nc.vector.memset(wu_x[:], 0.0)
            for _ in range(10):
                wu_ps = psum.tile([P, 512], F32, tag="ps", name="wu_ps")
                nc.tensor.matmul(wu_ps[:, :P], wu_w[:], wu_x[:], start=True, stop=True)

            # ---- all weight/input DMAs, merged + spread across queues ----
            x_in = [None] * n_blk

            def load_x(b, eng):
                nb, off = blocks[b], offs[b]
                t = acts.tile([P, 4, bmax], F8, tag="x", bufs=n_blk, name=f"x_{b}")
                eng.dma_start(t[:, :, :nb], dram["x_dr"][:, :, off : off + nb])
                x_in[b] = t

            def load_enc_w(name, k2s, eng):
                fi, fo = ENC_LAYERS[name]
                n_k2 = max(1, fi // 256)
                if name not in wsb:
                    wsb[name] = wpool.tile([P, n_k2, 2, fo], F8, tag=f"w_{name}", name=f"w_{name}")
                for k2 in k2s:
                    eng.dma_start(wsb[name][:, k2, :, :], dram["w_" + name][:, k2, :, :])

            def load_dec_w(name, eng):
                fi, fo = DEC_LAYERS[name]
                kp = min(P, fi)
                n_k = max(1, fi // P)
                t = wpool.tile([kp, n_k * fo], BF16, tag=f"w_{name}", name=f"w_{name}")
                eng.dma_start(t[:], dram["w_" + name][:])
                wsb[name] = t

            # prologue: first-MM deps first, then everything else
            load_x(0, nc.sync)
            load_enc_w("enc0", [0], nc.scalar)
            load_enc_w("enc0", [1], nc.scalar)
            for b in range(1, n_blk):
                load_x(b, nc.sync)
            load_enc_w("encu", [0, 1, 2, 3], nc.gpsimd)
            load_enc_w("enc2", [0, 1], nc.gpsimd)
            load_enc_w("mulv", [0], nc.gpsimd)

            bias_t = wpool.tile([P, sum(_bias_cols(n) for n in BIAS_ORDER)], F32, tag="biases", name="biases")
            nc.gpsimd.dma_start(bias_t[:], dram["biases"][:])
            b_off = {}
            o = 0
            for n in BIAS_ORDER:
                b_off[n] = o
                o += _bias_cols(n)

            eps_t = acts.tile([LAT, npad], F32, tag="eps", bufs=1, name="eps")
            nc.gpsimd.dma_start(eps_t[:], dram["epsT"][:])
            for name in DEC_LAYERS:
                load_dec_w(name, nc.gpsimd)

            def bias_ap(name, m, p0=0, p1=P):
                return bias_t[p0:p1, b_off[name] + m : b_off[name] + m + 1]

            # ---- activation tiles ----
            h0 = [[None] * 4 for _ in range(n_blk)]
            h1 = [[None] * 2 for _ in range(n_blk)]
            h2 = [None] * n_blk
            mu_t = [None] * n_blk
            sg_t = [None] * n_blk
            z_t = [None] * n_blk
            h3 = [[None] * 2 for _ in range(n_blk)]
            h4 = [[None] * 4 for _ in range(n_blk)]
            out_t = [None] * n_blk
            h5 = [[None] * 8 for _ in range(n_blk)]

            def mm_fp8(name, b, ins_of, evict):
                nb = blocks[b]
                fi, fo = ENC_LAYERS[name]
                w_t = wsb[name]
                n_k2 = max(1, fi // 256)
                n_m = max(1, fo // P)
                mp = min(P, fo)
                for m in range(n_m):
                    ps = psum.tile([P, 512], F32, tag="ps", name=f"ps_{name}_{m}_{b}")
                    for k2 in range(n_k2):
                        nc.tensor.matmul(
                            ps[:mp, :nb],
                            w_t[:, k2, :, m * mp : (m + 1) * mp],
                            ins_of(k2),
                            start=(k2 == 0),
                            stop=(k2 == n_k2 - 1),
                            perf_mode=DR,
                        )
                    evict(m, ps)

            def mm_bf16(name, b, ins, evict):
                nb = blocks[b]
                fi, fo = DEC_LAYERS[name]
                w_t = wsb[name]
                n_k = max(1, fi // P)
                n_m = max(1, fo // P)
                mp = min(P, fo)
                for m in range(n_m):
                    ps = psum.tile([P, 512], F32, tag="ps", name=f"ps_{name}_{m}_{b}")
                    for k in range(n_k):
                        nc.tensor.matmul(
                            ps[:mp, :nb],
                            w_t[:, k * fo + m * mp : k * fo + (m + 1) * mp],
                            ins[k][:, :nb],
                            start=(k == 0),
                            stop=(k == n_k - 1),
                        )
                    evict(m, ps)

            def relu_evict(eng, out_ap, ps_ap, b_ap):
                if eng is nc.scalar:
                    nc.scalar.activation(out_ap, ps_ap, AF.Relu, bias=b_ap, scale=1.0)
                else:
                    eng.tensor_scalar(out_ap, ps_ap, b_ap, 0.0, ALU.add, ALU.max)

            # ---- per-layer stages ----
            def enc0_stage(b):
                nb = blocks[b]
                xt = x_in[b]

                def ev(m, ps):
                    t = h0[b][m // 2]
                    if t is None:
                        t = acts.tile([P, 2, bmax], F8, tag=f"h0_{m // 2}", bufs=n_blk, name=f"h0_{m // 2}_{b}")
                        h0[b][m // 2] = t
                    eng = nc.scalar if m % 2 == 0 else nc.vector
                    relu_evict(eng, t[:, m % 2, :nb], ps[:, :nb], bias_ap("enc0", m))

                mm_fp8("enc0", b, lambda k2: xt[:, 2 * k2 : 2 * k2 + 2, :nb], ev)

            def encu_stage(b):
                nb = blocks[b]

                def ev(m, ps):
                    t = h1[b][m // 2]
                    if t is None:
                        t = acts.tile([P, 2, bmax], F8, tag=f"h1_{m // 2}", bufs=n_blk, name=f"h1_{m // 2}_{b}")
                        h1[b][m // 2] = t
                    eng = nc.scalar if m % 2 == 0 else nc.vector
                    relu_evict(eng, t[:, m % 2, :nb], ps[:, :nb], bias_ap("encu", m))

                mm_fp8("encu", b, lambda k2: h0[b][k2][:, :, :nb], ev)

            def enc2_stage(b):
                nb = blocks[b]

                def ev(m, ps):
                    t = h2[b]
                    if t is None:
                        t = acts.tile([P, 2, bmax], F8, tag="h2", bufs=n_blk, name=f"h2_{b}")
                        h2[b] = t
                    eng = nc.scalar if m % 2 == 0 else nc.vector
                    relu_evict(eng, t[:, m, :nb], ps[:, :nb], bias_ap("enc2", m))

                mm_fp8("enc2", b, lambda k2: h1[b][k2][:, :, :nb], ev)

            def mulv_stage(b):
                nb = blocks[b]

                def ev(m, ps):
                    mu = acts.tile([LAT, bmax], F32, tag="mu", bufs=2, name=f"mu_{b}")
                    sg = acts.tile([LAT, bmax], F32, tag="sg", bufs=2, name=f"sg_{b}")
                    nc.vector.tensor_scalar(
                        mu[:, :nb], ps[:LAT, :nb], 1.0 / (S_H * S_WM),
                        bias_ap("mulv", 0, 0, LAT), ALU.mult, ALU.add,
                    )
                    nc.scalar.activation(
                        sg[:, :nb], ps[LAT:, :nb], AF.Exp,
                        bias=bias_ap("mulv", 0, LAT, P), scale=0.5 / (S_H * S_WM),
                    )
                    mu_t[b], sg_t[b] = mu, sg

                mm_fp8("mulv", b, lambda k2: h2[b][:, :, :nb], ev)

            def lat_stage(b):
                nb, off = blocks[b], offs[b]
                tmp = acts.tile([LAT, bmax], F32, tag="tmp", bufs=2, name=f"tmp_{b}")
                nc.vector.tensor_mul(tmp[:, :nb], sg_t[b][:, :nb], eps_t[:, off : off + nb])
                z = acts.tile([LAT, bmax], BF16, tag="z", bufs=n_blk, name=f"z_{b}")
                nc.vector.tensor_add(z[:, :nb], tmp[:, :nb], mu_t[b][:, :nb])
                z_t[b] = z

            def dec0_stage(b):
                nb = blocks[b]

                def ev(m, ps):
                    t = acts.tile([P, bmax], BF16, tag=f"h3_{m}", bufs=n_blk, name=f"h3_{m}_{b}")
                    eng = nc.scalar if m % 2 == 0 else nc.vector
                    relu_evict(eng, t[:, :nb], ps[:, :nb], bias_ap("dec0", m))
                    h3[b][m] = t

                mm_bf16("dec0", b, [z_t[b]], ev)

            def dec1_stage(b):
                nb = blocks[b]

                def ev(m, ps):
                    t = acts.tile([P, bmax], BF16, tag=f"h4_{m}", bufs=n_blk, name=f"h4_{m}_{b}")
                    eng = nc.scalar if m % 2 == 0 else nc.vector
                    relu_evict(eng, t[:, :nb], ps[:, :nb], bias_ap("dec1", m))
                    h4[b][m] = t

                mm_bf16("dec1", b, h3[b], ev)

            def dec2_stage(b):
                nb = blocks[b]

                def ev(m, ps):
                    t = acts.tile([P, bmax], BF16, tag=f"h5_{m}", bufs=n_blk, name=f"h5_{m}_{b}")
                    eng = nc.scalar if m % 2 == 0 else nc.vector
                    relu_evict(eng, t[:, :nb], ps[:, :nb], bias_ap("dec2", m))
                    h5[b][m] = t

                mm_bf16("dec2", b, h4[b], ev)

            def fin_stage(b):
                nb, off = blocks[b], offs[b]
                ot = acts.tile([P, 4, bmax], F32, tag="out", bufs=2, name=f"out_{b}")
                out_t[b] = ot

                def ev(m, ps):
                    if m % 2 == 0:
                        nc.scalar.activation(
                            ot[:, m, :nb], ps[:, :nb], AF.Identity,
                            bias=bias_ap("fin", m), scale=1.0,
                        )
                    else:
                        nc.vector.tensor_scalar(
                            ot[:, m, :nb], ps[:, :nb], bias_ap("fin", m), None, ALU.add
                        )
                    nc.sync.dma_start(outT[:, m, off : off + nb], ot[:, m, :nb])

                mm_bf16("fin", b, h5[b], ev)

            # ---- schedule ----
            for b in range(n_blk):
                enc0_stage(b)
            # staggered software pipeline for everything after enc0
            def maybe(stage, b):
                if 0 <= b < n_blk:
                    stage(b)

            for i in range(n_blk + 7):
                maybe(encu_stage, i)
                maybe(enc2_stage, i - 2)
                maybe(mulv_stage, i - 3)
                maybe(lat_stage, i - 3)
                maybe(dec0_stage, i - 4)
                maybe(dec1_stage, i - 5)
                maybe(dec2_stage, i - 6)
                maybe(fin_stage, i - 7)

    nc.compile()
    return nc


def kernel(**inputs):
    x = np.asarray(inputs["x"], dtype=np.float32)
    lbl = np.asarray(inputs["cluster_labels"]).astype(np.int64)
    eps = np.asarray(inputs["eps"], dtype=np.float32)
    B = x.shape[0]

    counts = np.bincount(lbl, minlength=C)
    npad = max(512, _ceil_to(int(counts.max()), 64))
    blocks = _blocks_of(npad)

    rows = [np.nonzero(lbl == c)[0] for c in range(C)]

    mulv_W = np.concatenate([np.asarray(inputs["mu_W"]), np.asarray(inputs["lv_W"])], axis=1)
    mulv_b = np.concatenate([np.asarray(inputs["mu_b"]), 0.5 * np.asarray(inputs["lv_b"])])

    def bias_blob(per_cluster):
        cols = []
        for name in BIAS_ORDER:
            b = per_cluster[name]
            f = b.shape[0]
            if f >= P:
                cols.append(b.reshape(f // P, P).T)
            else:
                cols.append(np.tile(b.reshape(1, f).T, (P // f, 1)).reshape(P, 1))
        return np.ascontiguousarray(np.concatenate(cols, axis=1).astype(np.float32))

    shared_w = {
        "w_enc0": _w8(inputs["enc_W0"], S_W0),
        "w_enc2": _w8(inputs["enc_W2"], 1.0),
        "w_mulv": _w8(mulv_W, S_WM),
        "w_dec1": _wdec(inputs["dec_W1"]),
    }

    in_maps = []
    for c in range(C):
        r = rows[c]
        xT = np.zeros((D_IN, npad), np.float32)
        xT[:, : len(r)] = x[r].T
        x_dr = (xT * S_X).astype(F8_NP).reshape(4, P, npad).transpose(1, 0, 2)
        epsT = np.zeros((LAT, npad), np.float32)
        epsT[:, : len(r)] = eps[r].T
        m = dict(shared_w)
        m["x_dr"] = np.ascontiguousarray(x_dr)
        m["epsT"] = epsT
        m["w_encu"] = _w8(inputs["enc_Wu"][c], 1.0)
        m["w_dec0"] = _wdec(inputs["dec_Wu0"][c])
        m["w_dec2"] = _wdec(inputs["dec_Wu2"][c])
        m["w_fin"] = _wdec(inputs["fin_W"][c])
        m["biases"] = bias_blob({
            "enc0": S_H * np.asarray(inputs["enc_b0"]),
            "encu": S_H * np.asarray(inputs["enc_bu"][c]),
            "enc2": S_H * np.asarray(inputs["enc_b2"]),
            "mulv": mulv_b,
            "dec0": np.asarray(inputs["dec_bu0"][c]),
            "dec1": np.asarray(inputs["dec_b1"]),
            "dec2": np.asarray(inputs["dec_bu2"][c]),
            "fin": np.asarray(inputs["fin_b"][c]),
        })
        in_maps.append(m)

    nc = _build_module(npad, blocks)
    res = bass_utils.run_bass_kernel_spmd(nc, in_maps, core_ids=list(range(N_CORES)))
    global LAST_RESULTS
    LAST_RESULTS = res

    out = np.empty((B, D_IN), np.float32)
    for c in range(C):
        r = rows[c]
        o = res.results[c]["outT"]  # [128, 4, npad]
        out[r] = o.transpose(1, 0, 2).reshape(D_IN, npad)[:, : len(r)].T
    return out


# revision 10
# speedup vs baseline: 1.3048x; 1.0129x over previous
"""CISS-VAE (per-cluster MoE-routed MLP chain) Trainium2 kernel.

Strategy (routing on host, compute on device):
  - Rows are grouped by cluster label on the host; core c processes all rows
    of cluster c (C == n_cores == 8), so every GEMM is a dense per-cluster
    GEMM (no 8x redundant einsum like the reference).
  - The encoder (enc0, encu, enc2, fused mu|lv head) runs in fp8-e4m3 with
    DoubleRow matmuls (2 fp8 k-rows per cell, ~2x bf16 rate): the VAE latent
    z is dominated by the eps noise term, so encoder-side quantization error
    is attenuated far below the decoder's sensitivity (measured end-to-end
    rel err 1.9e-3 vs 1.8e-3 all-bf16). The decoder stays bf16 (decoder-side
    fp8 measured 1.2e-2..3.3e-2, too close to the 2e-2 gate).
  - fp8 scales chosen so every encoder PSUM eviction is scale-free
    (sigma_out == sigma_w * sigma_in): x*4, enc0 weights *16, hidden
    activations *64, encu/enc2 weights *1. Scale-free evictions are a
    2-ALU-op pattern (add bias, max 0) so they split across the Scalar AND
    Vector engines - eviction throughput, not the PE, limits the thin
    layers, so both engines share the drain work everywhere.
  - Schedule: enc0 runs as a plain layer-wavefront over the row blocks,
    then encu/enc2/mu|lv/latent/dec0/dec1/dec2/fin advance as one staggered
    software pipeline (layer L of block b interleaves with neighboring
    blocks' other layers), so thin stages always have thick matmuls in
    flight around them and the mu/sigma/z chain latency is hidden. npad is
    split into near-equal blocks (<=512 = one PSUM bank), largest first.
  - DMA descriptor issue costs ~0.65us of engine time each, so transfers
    are merged: one DMA per x block, per-k2 weight slabs, one merged
    [128, n_k*fo] tile per decoder layer, one bias blob, one output DMA per
    block. Issues spread across the sync/scalar/vector/gpsimd queues ahead
    of the eviction streams.
"""

import ml_dtypes
import numpy as np

import concourse.bacc as bacc
import concourse.mybir as mybir
import concourse.tile as tile
from concourse import bass_utils

P = 128
D_IN, LAT, C = 512, 64, 8
H0, H1, H2 = 1024, 512, 256
N_CORES = 8
F32 = mybir.dt.float32
BF16 = mybir.dt.bfloat16
F8 = mybir.dt.float8e4
AF = mybir.ActivationFunctionType
ALU = mybir.AluOpType
DR = mybir.MatmulPerfMode.DoubleRow
BF16_NP = ml_dtypes.bfloat16
F8_NP = ml_dtypes.float8_e4m3

S_X = 4.0    # fp8 scale on the x input
S_W0 = 16.0  # fp8 scale on enc0 weights
S_H = 64.0   # fp8 scale on encoder hidden activations (== S_X*S_W0 == 1*S_H)
S_WM = 16.0  # fp8 scale on the mu|lv head weights

ENC_LAYERS = dict(enc0=(D_IN, H0), encu=(H0, H1), enc2=(H1, H2), mulv=(H2, 2 * LAT))
DEC_LAYERS = dict(dec0=(LAT, H2), dec1=(H2, H1), dec2=(H1, H0), fin=(H0, D_IN))
BIAS_ORDER = ["enc0", "encu", "enc2", "mulv", "dec0", "dec1", "dec2", "fin"]


def _ceil_to(x, m):
    return ((x + m - 1) // m) * m


def _bias_cols(name):
    table = ENC_LAYERS if name in ENC_LAYERS else DEC_LAYERS
    return max(1, table[name][1] // P)


def _w8(w, scale):
    """[fi, fo] weight -> fp8 DR layout [128, n_k2, 2, fo]."""
    w = np.asarray(w, np.float32)
    fi, fo = w.shape
    n_k2 = max(1, fi // 256)
    a = (w * scale).astype(F8_NP).reshape(n_k2, 2, P, fo)
    return np.ascontiguousarray(a.transpose(2, 0, 1, 3))


def _wdec(w):
    """[fi, fo] bf16 weight -> merged [kp, n_k*fo] (k-slabs side by side)."""
    w = np.asarray(w, np.float32).astype(BF16_NP)
    fi, fo = w.shape
    if fi <= P:
        return np.ascontiguousarray(w)
    n_k = fi // P
    return np.ascontiguousarray(w.reshape(n_k, P, fo).transpose(1, 0, 2).reshape(P, n_k * fo))


def _blocks_of(npad):
    """Split npad into <=512 near-equal multiples of 64, descending."""
    n_blk = (npad + 511) // 512
    base = (npad // n_blk) // 64 * 64
    blocks = [base] * n_blk
    rem = npad - base * n_blk
    i = 0
    while rem > 0:
        blocks[i] += 64
        rem -= 64
        i = (i + 1) % n_blk
    return sorted(blocks, reverse=True)


def _build_module(npad, blocks):
    nc = bacc.Bacc("TRN2", target_bir_lowering=False, debug=False)
    n_blk = len(blocks)
    offs = [sum(blocks[:i]) for i in range(n_blk)]
    bmax = max(blocks)

    dram = {}

    def din(name, shape, dt):
        dram[name] = nc.dram_tensor(name, list(shape), dt, kind="ExternalInput").ap()
        return dram[name]

    din("x_dr", (P, 4, npad), F8)  # dim1 = k2*2 + grp
    din("epsT", (LAT, npad), F32)
    for name, (fi, fo) in ENC_LAYERS.items():
        din("w_" + name, (P, max(1, fi // 256), 2, fo), F8)
    for name, (fi, fo) in DEC_LAYERS.items():
        kp = min(P, fi)
        din("w_" + name, (kp, max(1, fi // P) * fo), BF16)
    n_bias = sum(_bias_cols(n) for n in BIAS_ORDER)
    din("biases", (P, n_bias), F32)
    outT = nc.dram_tensor("outT", [P, 4, npad], F32, kind="ExternalOutput").ap()

    with tile.TileContext(nc) as tc:
        with (
            tc.tile_pool(name="wpool", bufs=1) as wpool,
            tc.tile_pool(name="acts", bufs=2) as acts,
            tc.tile_pool(name="psum", bufs=8, space="PSUM") as psum,
        ):
            wsb = {}

            # ---- warm up the PE while engines/queues boot ----
            wu_w = wpool.tile([P, P], BF16, tag="wu_w", name="wu_w")
            wu_x = wpool.tile([P, P], BF16, tag="wu_x", name="wu_x")
            nc.vector.memset(wu_w[:], 0.0)
            nc.vector.memset(wu_x[:], 0.0)
            for _ in range(26):
                wu_ps = psum.tile([P, 512], F32, tag="ps", name="wu_ps")
                nc.tensor.matmul(wu_ps[:, :P], wu_w[:], wu_x[:], start=True, stop=True)

            # ---- all weight/input DMAs, merged + spread across queues ----
            x_in = [None] * n_blk

            def load_x(b, eng):
                nb, off = blocks[b], offs[b]
                t = acts.tile([P, 4, bmax], F8, tag="x", bufs=n_blk, name=f"x_{b}")
                eng.dma_start(t[:, :, :nb], dram["x_dr"][:, :, off : off + nb])
                x_in[b] = t

            def load_enc_w(name, k2s, eng):
                fi, fo = ENC_LAYERS[name]
                n_k2 = max(1, fi // 256)
                if name not in wsb:
                    wsb[name] = wpool.tile([P, n_k2, 2, fo], F8, tag=f"w_{name}", name=f"w_{name}")
                for k2 in k2s:
                    eng.dma_start(wsb[name][:, k2, :, :], dram["w_" + name][:, k2, :, :])

            def load_dec_w(name, eng):
                fi, fo = DEC_LAYERS[name]
                kp = min(P, fi)
                n_k = max(1, fi // P)
                t = wpool.tile([kp, n_k * fo], BF16, tag=f"w_{name}", name=f"w_{name}")
                eng.dma_start(t[:], dram["w_" + name][:])
                wsb[name] = t

            # prologue: first-MM deps first, then everything else
            load_x(0, nc.sync)
            load_enc_w("enc0", [0], nc.scalar)
            load_enc_w("enc0", [1], nc.scalar)
            for b in range(1, n_blk):
                load_x(b, nc.sync)
            load_enc_w("encu", [0, 1, 2, 3], nc.gpsimd)
            load_enc_w("enc2", [0, 1], nc.gpsimd)
            load_enc_w("mulv", [0], nc.gpsimd)

            bias_t = wpool.tile([P, sum(_bias_cols(n) for n in BIAS_ORDER)], F32, tag="biases", name="biases")
            nc.gpsimd.dma_start(bias_t[:], dram["biases"][:])
            b_off = {}
            o = 0
            for n in BIAS_ORDER:
                b_off[n] = o
                o += _bias_cols(n)

            load_dec_w("dec0", nc.gpsimd)
            eps_t = acts.tile([LAT, npad], F32, tag="eps", bufs=1, name="eps")
            nc.gpsimd.dma_start(eps_t[:], dram["epsT"][:])
            for name in ("dec1", "dec2", "fin"):
                load_dec_w(name, nc.gpsimd)

            def bias_ap(name, m, p0=0, p1=P):
                return bias_t[p0:p1, b_off[name] + m : b_off[name] + m + 1]

            # ---- activation tiles ----
            h0 = [[None] * 4 for _ in range(n_blk)]
            h1 = [[None] * 2 for _ in range(n_blk)]
            h2 = [None] * n_blk
            mu_t = [None] * n_blk
            sg_t = [None] * n_blk
            z_t = [None] * n_blk
            h3 = [[None] * 2 for _ in range(n_blk)]
            h4 = [[None] * 4 for _ in range(n_blk)]
            out_t = [None] * n_blk
            h5 = [[None] * 8 for _ in range(n_blk)]

            def mm_fp8(name, b, ins_of, evict):
                nb = blocks[b]
                fi, fo = ENC_LAYERS[name]
                w_t = wsb[name]
                n_k2 = max(1, fi // 256)
                n_m = max(1, fo // P)
                mp = min(P, fo)
                for m in range(n_m):
                    ps = psum.tile([P, 512], F32, tag="ps", name=f"ps_{name}_{m}_{b}")
                    for k2 in range(n_k2):
                        nc.tensor.matmul(
                            ps[:mp, :nb],
                            w_t[:, k2, :, m * mp : (m + 1) * mp],
                            ins_of(k2),
                            start=(k2 == 0),
                            stop=(k2 == n_k2 - 1),
                            perf_mode=DR,
                        )
                    evict(m, ps)

            def mm_bf16(name, b, ins, evict):
                nb = blocks[b]
                fi, fo = DEC_LAYERS[name]
                w_t = wsb[name]
                n_k = max(1, fi // P)
                n_m = max(1, fo // P)
                mp = min(P, fo)
                for m in range(n_m):
                    ps = psum.tile([P, 512], F32, tag="ps", name=f"ps_{name}_{m}_{b}")
                    for k in range(n_k):
                        nc.tensor.matmul(
                            ps[:mp, :nb],
                            w_t[:, k * fo + m * mp : k * fo + (m + 1) * mp],
                            ins[k][:, :nb],
                            start=(k == 0),
                            stop=(k == n_k - 1),
                        )
                    evict(m, ps)

            def relu_evict(eng, out_ap, ps_ap, b_ap):
                if eng is nc.scalar:
                    nc.scalar.activation(out_ap, ps_ap, AF.Relu, bias=b_ap, scale=1.0)
                else:
                    eng.tensor_scalar(out_ap, ps_ap, b_ap, 0.0, ALU.add, ALU.max)

            # ---- per-layer stages ----
            def enc0_stage(b):
                nb = blocks[b]
                xt = x_in[b]

                def ev(m, ps):
                    t = h0[b][m // 2]
                    if t is None:
                        t = acts.tile([P, 2, bmax], F8, tag=f"h0_{m // 2}", bufs=n_blk, name=f"h0_{m // 2}_{b}")
                        h0[b][m // 2] = t
                    eng = nc.scalar if m % 2 == 0 else nc.vector
                    relu_evict(eng, t[:, m % 2, :nb], ps[:, :nb], bias_ap("enc0", m))

                mm_fp8("enc0", b, lambda k2: xt[:, 2 * k2 : 2 * k2 + 2, :nb], ev)

            def encu_stage(b):
                nb = blocks[b]

                def ev(m, ps):
                    t = h1[b][m // 2]
                    if t is None:
                        t = acts.tile([P, 2, bmax], F8, tag=f"h1_{m // 2}", bufs=n_blk, name=f"h1_{m // 2}_{b}")
                        h1[b][m // 2] = t
                    eng = nc.scalar if m % 2 == 0 else nc.vector
                    relu_evict(eng, t[:, m % 2, :nb], ps[:, :nb], bias_ap("encu", m))

                mm_fp8("encu", b, lambda k2: h0[b][k2][:, :, :nb], ev)

            def enc2_stage(b):
                nb = blocks[b]

                def ev(m, ps):
                    t = h2[b]
                    if t is None:
                        t = acts.tile([P, 2, bmax], F8, tag="h2", bufs=n_blk, name=f"h2_{b}")
                        h2[b] = t
                    eng = nc.scalar if m % 2 == 0 else nc.vector
                    relu_evict(eng, t[:, m, :nb], ps[:, :nb], bias_ap("enc2", m))

                mm_fp8("enc2", b, lambda k2: h1[b][k2][:, :, :nb], ev)

            def mulv_stage(b):
                nb = blocks[b]

                def ev(m, ps):
                    mu = acts.tile([LAT, bmax], F32, tag="mu", bufs=2, name=f"mu_{b}")
                    sg = acts.tile([LAT, bmax], F32, tag="sg", bufs=2, name=f"sg_{b}")
                    nc.vector.tensor_scalar(
                        mu[:, :nb], ps[:LAT, :nb], 1.0 / (S_H * S_WM),
                        bias_ap("mulv", 0, 0, LAT), ALU.mult, ALU.add,
                    )
                    nc.scalar.activation(
                        sg[:, :nb], ps[LAT:, :nb], AF.Exp,
                        bias=bias_ap("mulv", 0, LAT, P), scale=0.5 / (S_H * S_WM),
                    )
                    mu_t[b], sg_t[b] = mu, sg

                mm_fp8("mulv", b, lambda k2: h2[b][:, :, :nb], ev)

            def lat_stage(b):
                nb, off = blocks[b], offs[b]
                tmp = acts.tile([LAT, bmax], F32, tag="tmp", bufs=2, name=f"tmp_{b}")
                nc.vector.tensor_mul(tmp[:, :nb], sg_t[b][:, :nb], eps_t[:, off : off + nb])
                z = acts.tile([LAT, bmax], BF16, tag="z", bufs=n_blk, name=f"z_{b}")
                nc.vector.tensor_add(z[:, :nb], tmp[:, :nb], mu_t[b][:, :nb])
                z_t[b] = z

            def dec0_stage(b):
                nb = blocks[b]

                def ev(m, ps):
                    t = acts.tile([P, bmax], BF16, tag=f"h3_{m}", bufs=n_blk, name=f"h3_{m}_{b}")
                    eng = nc.scalar if m % 2 == 0 else nc.vector
                    relu_evict(eng, t[:, :nb], ps[:, :nb], bias_ap("dec0", m))
                    h3[b][m] = t

                mm_bf16("dec0", b, [z_t[b]], ev)

            def dec1_stage(b):
                nb = blocks[b]

                def ev(m, ps):
                    t = acts.tile([P, bmax], BF16, tag=f"h4_{m}", bufs=n_blk, name=f"h4_{m}_{b}")
                    eng = nc.scalar if m % 2 == 0 else nc.vector
                    relu_evict(eng, t[:, :nb], ps[:, :nb], bias_ap("dec1", m))
                    h4[b][m] = t

                mm_bf16("dec1", b, h3[b], ev)

            def dec2_stage(b):
                nb = blocks[b]

                def ev(m, ps):
                    t = acts.tile([P, bmax], BF16, tag=f"h5_{m}", bufs=n_blk, name=f"h5_{m}_{b}")
                    eng = nc.scalar if m % 2 == 0 else nc.vector
                    relu_evict(eng, t[:, :nb], ps[:, :nb], bias_ap("dec2", m))
                    h5[b][m] = t

                mm_bf16("dec2", b, h4[b], ev)

            def fin_stage(b):
                nb, off = blocks[b], offs[b]
                ot = acts.tile([P, 4, bmax], F32, tag="out", bufs=2, name=f"out_{b}")
                out_t[b] = ot

                def ev(m, ps):
                    if m % 2 == 0:
                        nc.scalar.activation(
                            ot[:, m, :nb], ps[:, :nb], AF.Identity,
                            bias=bias_ap("fin", m), scale=1.0,
                        )
                    else:
                        nc.vector.tensor_scalar(
                            ot[:, m, :nb], ps[:, :nb], bias_ap("fin", m), None, ALU.add
                        )
                    nc.sync.dma_start(outT[:, m, off : off + nb], ot[:, m, :nb])

                mm_bf16("fin", b, h5[b], ev)

            # ---- schedule ----
            for b in range(n_blk):
                enc0_stage(b)
            # staggered software pipeline for everything after enc0
            def maybe(stage, b):
                if 0 <= b < n_blk:
                    stage(b)

            for i in range(n_blk + 6):
                maybe(encu_stage, i)
                maybe(enc2_stage, i - 1)
                maybe(mulv_stage, i - 2)
                maybe(lat_stage, i - 2)
                maybe(dec0_stage, i - 3)
                maybe(dec1_stage, i - 4)
                maybe(dec2_stage, i - 5)
                maybe(fin_stage, i - 6)

    nc.compile()
    return nc


def kernel(**inputs):
    x = np.asarray(inputs["x"], dtype=np.float32)
    lbl = np.asarray(inputs["cluster_labels"]).astype(np.int64)
    eps = np.asarray(inputs["eps"], dtype=np.float32)
    B = x.shape[0]

    counts = np.bincount(lbl, minlength=C)
    npad = max(512, _ceil_to(int(counts.max()), 64))
    blocks = _blocks_of(npad)

    rows = [np.nonzero(lbl == c)[0] for c in range(C)]

    mulv_W = np.concatenate([np.asarray(inputs["mu_W"]), np.asarray(inputs["lv_W"])], axis=1)
    mulv_b = np.concatenate([np.asarray(inputs["mu_b"]), 0.5 * np.asarray(inputs["lv_b"])])

    def bias_blob(per_cluster):
        cols = []
        for name in BIAS_ORDER:
            b = per_cluster[name]
            f = b.shape[0]
            if f >= P:
                cols.append(b.reshape(f // P, P).T)
            else:
                cols.append(np.tile(b.reshape(1, f).T, (P // f, 1)).reshape(P, 1))
        return np.ascontiguousarray(np.concatenate(cols, axis=1).astype(np.float32))

    shared_w = {
        "w_enc0": _w8(inputs["enc_W0"], S_W0),
        "w_enc2": _w8(inputs["enc_W2"], 1.0),
        "w_mulv": _w8(mulv_W, S_WM),
        "w_dec1": _wdec(inputs["dec_W1"]),
    }

    in_maps = []
    for c in range(C):
        r = rows[c]
        xT = np.zeros((D_IN, npad), np.float32)
        xT[:, : len(r)] = x[r].T
        x_dr = (xT * S_X).astype(F8_NP).reshape(4, P, npad).transpose(1, 0, 2)
        epsT = np.zeros((LAT, npad), np.float32)
        epsT[:, : len(r)] = eps[r].T
        m = dict(shared_w)
        m["x_dr"] = np.ascontiguousarray(x_dr)
        m["epsT"] = epsT
        m["w_encu"] = _w8(inputs["enc_Wu"][c], 1.0)
        m["w_dec0"] = _wdec(inputs["dec_Wu0"][c])
        m["w_dec2"] = _wdec(inputs["dec_Wu2"][c])
        m["w_fin"] = _wdec(inputs["fin_W"][c])
        m["biases"] = bias_blob({
            "enc0": S_H * np.asarray(inputs["enc_b0"]),
            "encu": S_H * np.asarray(inputs["enc_bu"][c]),
            "enc2": S_H * np.asarray(inputs["enc_b2"]),
            "mulv": mulv_b,
            "dec0": np.asarray(inputs["dec_bu0"][c]),
            "dec1": np.asarray(inputs["dec_b1"]),
            "dec2": np.asarray(inputs["dec_bu2"][c]),
            "fin": np.asarray(inputs["fin_b"][c]),
        })
        in_maps.append(m)

    nc = _build_module(npad, blocks)
    res = bass_utils.run_bass_kernel_spmd(nc, in_maps, core_ids=list(range(N_CORES)))
    global LAST_RESULTS
    LAST_RESULTS = res

    out = np.empty((B, D_IN), np.float32)
    for c in range(C):
        r = rows[c]
        o = res.results[c]["outT"]  # [128, 4, npad]
        out[r] = o.transpose(1, 0, 2).reshape(D_IN, npad)[:, : len(r)].T
    return out


# revision 12
# speedup vs baseline: 1.3119x; 1.0054x over previous
"""CISS-VAE (per-cluster MoE-routed MLP chain) Trainium2 kernel.

Strategy (routing on host, compute on device):
  - Rows are grouped by cluster label on the host; core c processes all rows
    of cluster c (C == n_cores == 8), so every GEMM is a dense per-cluster
    GEMM (no 8x redundant einsum like the reference).
  - The encoder (enc0, encu, enc2, fused mu|lv head) runs in fp8-e4m3 with
    DoubleRow matmuls (2 fp8 k-rows per cell, ~2x bf16 rate): the VAE latent
    z is dominated by the eps noise term, so encoder-side quantization error
    is attenuated far below the decoder's sensitivity (measured end-to-end
    rel err 1.9e-3 vs 1.8e-3 all-bf16). The decoder stays bf16 (decoder-side
    fp8 measured 1.2e-2..3.3e-2, too close to the 2e-2 gate).
  - fp8 scales chosen so every encoder PSUM eviction is scale-free
    (sigma_out == sigma_w * sigma_in): x*4, enc0 weights *16, hidden
    activations *64, encu/enc2 weights *1. Scale-free evictions are a
    2-ALU-op pattern (add bias, max 0) so they split across the Scalar AND
    Vector engines - eviction throughput, not the PE, limits the thin
    layers, so both engines share the drain work everywhere.
  - Schedule: enc0 runs as a plain layer-wavefront over the row blocks,
    then encu/enc2/mu|lv/latent/dec0/dec1/dec2/fin advance as one staggered
    software pipeline (layer L of block b interleaves with neighboring
    blocks' other layers), so thin stages always have thick matmuls in
    flight around them and the mu/sigma/z chain latency is hidden. npad is
    split into near-equal blocks (<=512 = one PSUM bank), largest first.
  - DMA descriptor issue costs ~0.65us of engine time each, so transfers
    are merged: one DMA per x block, per-k2 weight slabs, one merged
    [128, n_k*fo] tile per decoder layer, one bias blob, one output DMA per
    block. Issues spread across the sync/scalar/vector/gpsimd queues ahead
    of the eviction streams.
"""

import ml_dtypes
import numpy as np

import concourse.bacc as bacc
import concourse.mybir as mybir
import concourse.tile as tile
from concourse import bass_utils

P = 128
D_IN, LAT, C = 512, 64, 8
H0, H1, H2 = 1024, 512, 256
N_CORES = 8
F32 = mybir.dt.float32
BF16 = mybir.dt.bfloat16
F8 = mybir.dt.float8e4
AF = mybir.ActivationFunctionType
ALU = mybir.AluOpType
DR = mybir.MatmulPerfMode.DoubleRow
BF16_NP = ml_dtypes.bfloat16
F8_NP = ml_dtypes.float8_e4m3

S_X = 4.0    # fp8 scale on the x input
S_W0 = 16.0  # fp8 scale on enc0 weights
S_H = 64.0   # fp8 scale on encoder hidden activations (== S_X*S_W0 == 1*S_H)
S_WM = 16.0  # fp8 scale on the mu|lv head weights

ENC_LAYERS = dict(enc0=(D_IN, H0), encu=(H0, H1), enc2=(H1, H2), mulv=(H2, 2 * LAT))
DEC_LAYERS = dict(dec0=(LAT, H2), dec1=(H2, H1), dec2=(H1, H0), fin=(H0, D_IN))
BIAS_ORDER = ["enc0", "encu", "enc2", "mulv", "dec0", "dec1", "dec2", "fin"]


def _ceil_to(x, m):
    return ((x + m - 1) // m) * m


def _bias_cols(name):
    table = ENC_LAYERS if name in ENC_LAYERS else DEC_LAYERS
    return max(1, table[name][1] // P)


def _w8(w, scale):
    """[fi, fo] weight -> fp8 DR layout [128, n_k2, 2, fo]."""
    w = np.asarray(w, np.float32)
    fi, fo = w.shape
    n_k2 = max(1, fi // 256)
    a = (w * scale).astype(F8_NP).reshape(n_k2, 2, P, fo)
    return np.ascontiguousarray(a.transpose(2, 0, 1, 3))


def _wdec(w):
    """[fi, fo] bf16 weight -> merged [kp, n_k*fo] (k-slabs side by side)."""
    w = np.asarray(w, np.float32).astype(BF16_NP)
    fi, fo = w.shape
    if fi <= P:
        return np.ascontiguousarray(w)
    n_k = fi // P
    return np.ascontiguousarray(w.reshape(n_k, P, fo).transpose(1, 0, 2).reshape(P, n_k * fo))


def _blocks_of(npad):
    """Split npad into <=512 near-equal multiples of 64, descending."""
    n_blk = (npad + 511) // 512
    base = (npad // n_blk) // 64 * 64
    blocks = [base] * n_blk
    rem = npad - base * n_blk
    i = 0
    while rem > 0:
        blocks[i] += 64
        rem -= 64
        i = (i + 1) % n_blk
    return sorted(blocks, reverse=True)


def _build_module(npad, blocks):
    nc = bacc.Bacc("TRN2", target_bir_lowering=False, debug=False)
    n_blk = len(blocks)
    offs = [sum(blocks[:i]) for i in range(n_blk)]
    bmax = max(blocks)

    dram = {}

    def din(name, shape, dt):
        dram[name] = nc.dram_tensor(name, list(shape), dt, kind="ExternalInput").ap()
        return dram[name]

    din("x_dr", (P, 4, npad), F8)  # dim1 = k2*2 + grp
    din("epsT", (LAT, npad), F32)
    for name, (fi, fo) in ENC_LAYERS.items():
        din("w_" + name, (P, max(1, fi // 256), 2, fo), F8)
    for name, (fi, fo) in DEC_LAYERS.items():
        kp = min(P, fi)
        din("w_" + name, (kp, max(1, fi // P) * fo), BF16)
    n_bias = sum(_bias_cols(n) for n in BIAS_ORDER)
    din("biases", (P, n_bias), F32)
    outT = nc.dram_tensor("outT", [P, 4, npad], F32, kind="ExternalOutput").ap()

    with tile.TileContext(nc) as tc:
        with (
            tc.tile_pool(name="wpool", bufs=1) as wpool,
            tc.tile_pool(name="acts", bufs=2) as acts,
            tc.tile_pool(name="psum", bufs=8, space="PSUM") as psum,
        ):
            wsb = {}

            # ---- warm up the PE while engines/queues boot ----
            wu_w = wpool.tile([P, P], BF16, tag="wu_w", name="wu_w")
            wu_x = wpool.tile([P, P], BF16, tag="wu_x", name="wu_x")
            nc.vector.memset(wu_w[:], 0.0)
            nc.vector.memset(wu_x[:], 0.0)
            for _ in range(26):
                wu_ps = psum.tile([P, 512], F32, tag="ps", name="wu_ps")
                nc.tensor.matmul(wu_ps[:, :P], wu_w[:], wu_x[:], start=True, stop=True)

            # ---- all weight/input DMAs, merged + spread across queues ----
            x_in = [None] * n_blk

            def load_x(b, eng):
                nb, off = blocks[b], offs[b]
                t = acts.tile([P, 4, bmax], F8, tag="x", bufs=n_blk, name=f"x_{b}")
                eng.dma_start(t[:, :, :nb], dram["x_dr"][:, :, off : off + nb])
                x_in[b] = t

            def load_enc_w(name, k2s, eng):
                fi, fo = ENC_LAYERS[name]
                n_k2 = max(1, fi // 256)
                if name not in wsb:
                    wsb[name] = wpool.tile([P, n_k2, 2, fo], F8, tag=f"w_{name}", name=f"w_{name}")
                for k2 in k2s:
                    eng.dma_start(wsb[name][:, k2, :, :], dram["w_" + name][:, k2, :, :])

            def load_dec_w(name, eng):
                fi, fo = DEC_LAYERS[name]
                kp = min(P, fi)
                n_k = max(1, fi // P)
                t = wpool.tile([kp, n_k * fo], BF16, tag=f"w_{name}", name=f"w_{name}")
                eng.dma_start(t[:], dram["w_" + name][:])
                wsb[name] = t

            # prologue: first-MM deps first. x rides ahead of the decoder
            # flood on the sync HWDGE queue (in-order per queue); enc0
            # weights + bias blob on scalar (before its eviction stream);
            # the other encoder weights on the gpsimd SWDGE engine.
            load_x(0, nc.sync)
            load_enc_w("enc0", [0], nc.scalar)
            load_enc_w("enc0", [1], nc.scalar)
            bias_t = wpool.tile([P, sum(_bias_cols(n) for n in BIAS_ORDER)], F32, tag="biases", name="biases")
            nc.scalar.dma_start(bias_t[:], dram["biases"][:])
            b_off = {}
            o = 0
            for n in BIAS_ORDER:
                b_off[n] = o
                o += _bias_cols(n)
            for b in range(1, n_blk):
                load_x(b, nc.sync)
            load_enc_w("encu", [0, 1, 2, 3], nc.gpsimd)
            load_enc_w("enc2", [0, 1], nc.gpsimd)
            load_enc_w("mulv", [0], nc.gpsimd)
            load_dec_w("dec0", nc.gpsimd)
            eps_t = acts.tile([LAT, npad], F32, tag="eps", bufs=1, name="eps")
            nc.sync.dma_start(eps_t[:], dram["epsT"][:])
            for name in ("dec1", "dec2", "fin"):
                load_dec_w(name, nc.sync)

            def bias_ap(name, m, p0=0, p1=P):
                return bias_t[p0:p1, b_off[name] + m : b_off[name] + m + 1]

            # ---- activation tiles ----
            h0 = [[None] * 4 for _ in range(n_blk)]
            h1 = [[None] * 2 for _ in range(n_blk)]
            h2 = [None] * n_blk
            mu_t = [None] * n_blk
            sg_t = [None] * n_blk
            z_t = [None] * n_blk
            h3 = [[None] * 2 for _ in range(n_blk)]
            h4 = [[None] * 4 for _ in range(n_blk)]
            out_t = [None] * n_blk
            h5 = [[None] * 8 for _ in range(n_blk)]

            def mm_fp8(name, b, ins_of, evict):
                nb = blocks[b]
                fi, fo = ENC_LAYERS[name]
                w_t = wsb[name]
                n_k2 = max(1, fi // 256)
                n_m = max(1, fo // P)
                mp = min(P, fo)
                for m in range(n_m):
                    ps = psum.tile([P, 512], F32, tag="ps", name=f"ps_{name}_{m}_{b}")
                    for k2 in range(n_k2):
                        nc.tensor.matmul(
                            ps[:mp, :nb],
                            w_t[:, k2, :, m * mp : (m + 1) * mp],
                            ins_of(k2),
                            start=(k2 == 0),
                            stop=(k2 == n_k2 - 1),
                            perf_mode=DR,
                        )
                    evict(m, ps)

            def mm_bf16(name, b, ins, evict):
                nb = blocks[b]
                fi, fo = DEC_LAYERS[name]
                w_t = wsb[name]
                n_k = max(1, fi // P)
                n_m = max(1, fo // P)
                mp = min(P, fo)
                for m in range(n_m):
                    ps = psum.tile([P, 512], F32, tag="ps", name=f"ps_{name}_{m}_{b}")
                    for k in range(n_k):
                        nc.tensor.matmul(
                            ps[:mp, :nb],
                            w_t[:, k * fo + m * mp : k * fo + (m + 1) * mp],
                            ins[k][:, :nb],
                            start=(k == 0),
                            stop=(k == n_k - 1),
                        )
                    evict(m, ps)

            def relu_evict(eng, out_ap, ps_ap, b_ap):
                if eng is nc.scalar:
                    nc.scalar.activation(out_ap, ps_ap, AF.Relu, bias=b_ap, scale=1.0)
                else:
                    eng.tensor_scalar(out_ap, ps_ap, b_ap, 0.0, ALU.add, ALU.max)

            # ---- per-layer stages ----
            def enc0_stage(b):
                nb = blocks[b]
                xt = x_in[b]

                def ev(m, ps):
                    t = h0[b][m // 2]
                    if t is None:
                        t = acts.tile([P, 2, bmax], F8, tag=f"h0_{m // 2}", bufs=n_blk, name=f"h0_{m // 2}_{b}")
                        h0[b][m // 2] = t
                    eng = nc.scalar if m % 2 == 0 else nc.vector
                    relu_evict(eng, t[:, m % 2, :nb], ps[:, :nb], bias_ap("enc0", m))

                mm_fp8("enc0", b, lambda k2: xt[:, 2 * k2 : 2 * k2 + 2, :nb], ev)

            def encu_stage(b):
                nb = blocks[b]

                def ev(m, ps):
                    t = h1[b][m // 2]
                    if t is None:
                        t = acts.tile([P, 2, bmax], F8, tag=f"h1_{m // 2}", bufs=n_blk, name=f"h1_{m // 2}_{b}")
                        h1[b][m // 2] = t
                    eng = nc.scalar if m % 2 == 0 else nc.vector
                    relu_evict(eng, t[:, m % 2, :nb], ps[:, :nb], bias_ap("encu", m))

                mm_fp8("encu", b, lambda k2: h0[b][k2][:, :, :nb], ev)

            def enc2_stage(b):
                nb = blocks[b]

                def ev(m, ps):
                    t = h2[b]
                    if t is None:
                        t = acts.tile([P, 2, bmax], F8, tag="h2", bufs=n_blk, name=f"h2_{b}")
                        h2[b] = t
                    eng = nc.scalar if m % 2 == 0 else nc.vector
                    relu_evict(eng, t[:, m, :nb], ps[:, :nb], bias_ap("enc2", m))

                mm_fp8("enc2", b, lambda k2: h1[b][k2][:, :, :nb], ev)

            def mulv_stage(b):
                nb = blocks[b]

                def ev(m, ps):
                    mu = acts.tile([LAT, bmax], F32, tag="mu", bufs=2, name=f"mu_{b}")
                    sg = acts.tile([LAT, bmax], F32, tag="sg", bufs=2, name=f"sg_{b}")
                    nc.vector.tensor_scalar(
                        mu[:, :nb], ps[:LAT, :nb], 1.0 / (S_H * S_WM),
                        bias_ap("mulv", 0, 0, LAT), ALU.mult, ALU.add,
                    )
                    nc.scalar.activation(
                        sg[:, :nb], ps[LAT:, :nb], AF.Exp,
                        bias=bias_ap("mulv", 0, LAT, P), scale=0.5 / (S_H * S_WM),
                    )
                    mu_t[b], sg_t[b] = mu, sg

                mm_fp8("mulv", b, lambda k2: h2[b][:, :, :nb], ev)

            def lat_stage(b):
                nb, off = blocks[b], offs[b]
                tmp = acts.tile([LAT, bmax], F32, tag="tmp", bufs=2, name=f"tmp_{b}")
                nc.vector.tensor_mul(tmp[:, :nb], sg_t[b][:, :nb], eps_t[:, off : off + nb])
                z = acts.tile([LAT, bmax], BF16, tag="z", bufs=n_blk, name=f"z_{b}")
                nc.vector.tensor_add(z[:, :nb], tmp[:, :nb], mu_t[b][:, :nb])
                z_t[b] = z

            def dec0_stage(b):
                nb = blocks[b]

                def ev(m, ps):
                    t = acts.tile([P, bmax], BF16, tag=f"h3_{m}", bufs=n_blk, name=f"h3_{m}_{b}")
                    eng = nc.scalar if m % 2 == 0 else nc.vector
                    relu_evict(eng, t[:, :nb], ps[:, :nb], bias_ap("dec0", m))
                    h3[b][m] = t

                mm_bf16("dec0", b, [z_t[b]], ev)

            def dec1_stage(b):
                nb = blocks[b]

                def ev(m, ps):
                    t = acts.tile([P, bmax], BF16, tag=f"h4_{m}", bufs=n_blk, name=f"h4_{m}_{b}")
                    eng = nc.scalar if m % 2 == 0 else nc.vector
                    relu_evict(eng, t[:, :nb], ps[:, :nb], bias_ap("dec1", m))
                    h4[b][m] = t

                mm_bf16("dec1", b, h3[b], ev)

            def dec2_stage(b):
                nb = blocks[b]

                def ev(m, ps):
                    t = acts.tile([P, bmax], BF16, tag=f"h5_{m}", bufs=n_blk, name=f"h5_{m}_{b}")
                    eng = nc.scalar if m % 2 == 0 else nc.vector
                    relu_evict(eng, t[:, :nb], ps[:, :nb], bias_ap("dec2", m))
                    h5[b][m] = t

                mm_bf16("dec2", b, h4[b], ev)

            def fin_stage(b):
                nb, off = blocks[b], offs[b]
                ot = acts.tile([P, 4, bmax], F32, tag="out", bufs=2, name=f"out_{b}")
                out_t[b] = ot

                def ev(m, ps):
                    if m % 2 == 0:
                        nc.scalar.activation(
                            ot[:, m, :nb], ps[:, :nb], AF.Identity,
                            bias=bias_ap("fin", m), scale=1.0,
                        )
                    else:
                        nc.vector.tensor_scalar(
                            ot[:, m, :nb], ps[:, :nb], bias_ap("fin", m), None, ALU.add
                        )
                    nc.sync.dma_start(outT[:, m, off : off + nb], ot[:, m, :nb])

                mm_bf16("fin", b, h5[b], ev)

            # ---- schedule ----
            for b in range(n_blk):
                enc0_stage(b)
            # staggered software pipeline for everything after enc0
            def maybe(stage, b):
                if 0 <= b < n_blk:
                    stage(b)

            for i in range(n_blk + 6):
                maybe(encu_stage, i)
                maybe(enc2_stage, i - 1)
                maybe(mulv_stage, i - 2)
                maybe(lat_stage, i - 2)
                maybe(dec0_stage, i - 3)
                maybe(dec1_stage, i - 4)
                maybe(dec2_stage, i - 5)
                maybe(fin_stage, i - 6)

    nc.compile()
    return nc


def kernel(**inputs):
    x = np.asarray(inputs["x"], dtype=np.float32)
    lbl = np.asarray(inputs["cluster_labels"]).astype(np.int64)
    eps = np.asarray(inputs["eps"], dtype=np.float32)
    B = x.shape[0]

    counts = np.bincount(lbl, minlength=C)
    npad = max(512, _ceil_to(int(counts.max()), 64))
    blocks = _blocks_of(npad)

    rows = [np.nonzero(lbl == c)[0] for c in range(C)]

    mulv_W = np.concatenate([np.asarray(inputs["mu_W"]), np.asarray(inputs["lv_W"])], axis=1)
    mulv_b = np.concatenate([np.asarray(inputs["mu_b"]), 0.5 * np.asarray(inputs["lv_b"])])

    def bias_blob(per_cluster):
        cols = []
        for name in BIAS_ORDER:
            b = per_cluster[name]
            f = b.shape[0]
            if f >= P:
                cols.append(b.reshape(f // P, P).T)
            else:
                cols.append(np.tile(b.reshape(1, f).T, (P // f, 1)).reshape(P, 1))
        return np.ascontiguousarray(np.concatenate(cols, axis=1).astype(np.float32))

    shared_w = {
        "w_enc0": _w8(inputs["enc_W0"], S_W0),
        "w_enc2": _w8(inputs["enc_W2"], 1.0),
        "w_mulv": _w8(mulv_W, S_WM),
        "w_dec1": _wdec(inputs["dec_W1"]),
    }

    in_maps = []
    for c in range(C):
        r = rows[c]
        xT = np.zeros((D_IN, npad), np.float32)
        xT[:, : len(r)] = x[r].T
        x_dr = (xT * S_X).astype(F8_NP).reshape(4, P, npad).transpose(1, 0, 2)
        epsT = np.zeros((LAT, npad), np.float32)
        epsT[:, : len(r)] = eps[r].T
        m = dict(shared_w)
        m["x_dr"] = np.ascontiguousarray(x_dr)
        m["epsT"] = epsT
        m["w_encu"] = _w8(inputs["enc_Wu"][c], 1.0)
        m["w_dec0"] = _wdec(inputs["dec_Wu0"][c])
        m["w_dec2"] = _wdec(inputs["dec_Wu2"][c])
        m["w_fin"] = _wdec(inputs["fin_W"][c])
        m["biases"] = bias_blob({
            "enc0": S_H * np.asarray(inputs["enc_b0"]),
            "encu": S_H * np.asarray(inputs["enc_bu"][c]),
            "enc2": S_H * np.asarray(inputs["enc_b2"]),
            "mulv": mulv_b,
            "dec0": np.asarray(inputs["dec_bu0"][c]),
            "dec1": np.asarray(inputs["dec_b1"]),
            "dec2": np.asarray(inputs["dec_bu2"][c]),
            "fin": np.asarray(inputs["fin_b"][c]),
        })
        in_maps.append(m)

    nc = _build_module(npad, blocks)
    res = bass_utils.run_bass_kernel_spmd(nc, in_maps, core_ids=list(range(N_CORES)))
    global LAST_RESULTS
    LAST_RESULTS = res

    out = np.empty((B, D_IN), np.float32)
    for c in range(C):
        r = rows[c]
        o = res.results[c]["outT"]  # [128, 4, npad]
        out[r] = o.transpose(1, 0, 2).reshape(D_IN, npad)[:, : len(r)].T
    return out
